# revision 1
# baseline (speedup 1.0000x reference)
"""MultiHeadAttention Trainium2 kernel.

Sharding: 8 cores = 4 batches x 2 head-groups (8 heads each).
Each core computes, for its (batch b, head-group g):
  Q^T = Wq_g @ Xq^T, K^T = Wk_g @ Xk^T   (f32r matmuls, [headdim, S] layout)
  V   = Xv @ Wv_g^T                       ([S, 512] layout, +ones col, mask-scaled)
  scores^T[k,q] per head (K=64 matmuls), e = exp(s/8) on ACT (PSUM->SBUF)
  x~^T/sums via [V|1]-stationary matmul (M=65), normalize via reciprocal +
  gpsimd partition_broadcast, out^T_partial = Wo_g^T.T @ x^T.
Host sums the two head-group partials per batch and transposes back.

Mask handling: V rows and the ones column are multiplied by mask (0/1), which
masks both the attnV numerator and the softmax denominator exactly.
"""
import contextlib
import os

import numpy as np
import concourse.bass as bass  # noqa: F401
import concourse.tile as tile
from concourse import bacc, mybir
from concourse.bass_utils import run_bass_kernel_spmd

F32 = mybir.dt.float32
F32R = mybir.dt.float32r
EXP = mybir.ActivationFunctionType.Exp

B, S, DM = 4, 2048, 1024
H = 16
DK = 64
HLOC = 8              # heads per core
CW = HLOC * DK        # 512 local head dims per core
NC_CORES = 8
KT = S // 128         # 16 k-tiles
NB = S // 512         # 4 q/s blocks of 512
MT = CW // 128        # 4 m-tiles of local head dims
DT = DM // 128        # 8 contraction tiles over d_model
SCALE = 1.0 / np.sqrt(DK)

_NC = None


def _env(k, d):
    return int(os.environ.get(k, d))


def _build():
    nc = bacc.Bacc()
    xqT = nc.dram_tensor("xqT", [DM, S], F32, kind="ExternalInput")
    xkT = nc.dram_tensor("xkT", [DM, S], F32, kind="ExternalInput")
    xvT = nc.dram_tensor("xvT", [DM, S], F32, kind="ExternalInput")
    wqT = nc.dram_tensor("wqT", [DM, CW], F32, kind="ExternalInput")
    wkT = nc.dram_tensor("wkT", [DM, CW], F32, kind="ExternalInput")
    wvT = nc.dram_tensor("wvT", [DM, CW], F32, kind="ExternalInput")
    woT = nc.dram_tensor("woT", [CW, DM], F32, kind="ExternalInput")
    maskf = nc.dram_tensor("maskf", [128, KT], F32, kind="ExternalInput")
    outT = nc.dram_tensor("outT", [DM, S], F32, kind="ExternalOutput")

    with tile.TileContext(nc) as tc, contextlib.ExitStack() as ctx:
        persist = ctx.enter_context(tc.tile_pool(name="persist", bufs=1))

        # --- persistent tiles: mask, wo, Q^T/K^T slices, V ---
        m_sb = persist.tile([128, KT], F32)
        nc.sync.dma_start(m_sb[:], maskf[:])
        ones8 = persist.tile([128, HLOC], F32)
        nc.vector.memset(ones8[:], 1.0)
        warm = persist.tile([1, 1], F32)
        nc.scalar.activation(warm[:], ones8[0:1, 0:1], EXP, scale=1.0)
        q_tiles = {}   # (m, nb) -> [128, 512] f32r  (Q^T slice)
        k_tiles = {}
        for m in range(MT):
            for n in range(NB):
                q_tiles[(m, n)] = persist.tile(
                    [128, 512], F32R, tag=f"q{m}_{n}", name=f"q{m}_{n}")
                k_tiles[(m, n)] = persist.tile(
                    [128, 512], F32R, tag=f"k{m}_{n}", name=f"k{m}_{n}")
        v_sb = persist.tile([128, KT, HLOC, DK + 1], F32R, tag="v")

        # ---------------- Phase A: projections ----------------
        wq_pool = ctx.enter_context(tc.tile_pool(name="wqp", bufs=1))
        xt = ctx.enter_context(tc.tile_pool(name="xt", bufs=_env("K_XT_BUFS", 8)))
        ctxA = contextlib.ExitStack()
        with ctxA:
            wkv_pool = ctxA.enter_context(tc.tile_pool(name="wkv", bufs=1))
            psA = ctxA.enter_context(tc.tile_pool(name="psA", bufs=8, space="PSUM"))
            wq_sb = [wq_pool.tile([128, CW], F32R, tag=f"wq{k}", name=f"wq{k}")
                     for k in range(DT)]
            wk_sb = [wkv_pool.tile([128, CW], F32R, tag=f"wk{k}", name=f"wk{k}")
                     for k in range(DT)]
            wv_sb = [wq_pool.tile([128, CW], F32R, tag=f"wv{k}", name=f"wv{k}")
                     for k in range(DT)]
            wo_t = [persist.tile([128, DM], F32R, tag=f"wo{k}", name=f"wo{k}")
                    for k in range(MT)]

            def w_dma(tiles, src, k):
                nc.sync.dma_start(
                    tiles[k][:],
                    src[k * 128:(k + 1) * 128, :].bitcast(F32R))

            def dma_block(src, n, nm, wtiles=None, wsrc=None):
                tiles = [xt.tile([128, 512], F32R, tag="xt",
                                 name=f"{nm}{n}_{i}") for i in range(DT)]
                for k in range(DT):
                    if wtiles is not None:
                        w_dma(wtiles, wsrc, k)
                    nc.sync.dma_start(
                        tiles[k][:],
                        src[k * 128:(k + 1) * 128,
                            n * 512:(n + 1) * 512].bitcast(F32R))
                return tiles

            def proj_group(dst_tiles, w_sb, xts, n, m, pool, tag):
                ps = pool.tile([128, 512], F32, tag=tag, name=f"pj{n}_{m}_{tag}")
                for k in range(DT):
                    nc.tensor.matmul(
                        ps[:], w_sb[k][:, m * 128:(m + 1) * 128],
                        xts[k][:], start=(k == 0), stop=(k == DT - 1))
                nc.vector.tensor_copy(dst_tiles[(m, n)][:], ps[:])

            def proj_block(dst_tiles, w_sb, src, n, nm, wsrc=None):
                xts = dma_block(src, n, nm,
                                wtiles=w_sb if wsrc is not None else None,
                                wsrc=wsrc)
                for m in range(MT):
                    proj_group(dst_tiles, w_sb, xts, n, m, psA, "pa")

            def v_group(n, sm, xts, pool, tag):
                t = n * 4 + sm
                ps = pool.tile([128, 512], F32, tag=tag, name=f"vps{n}_{sm}")
                for k in range(DT):
                    nc.tensor.matmul(
                        ps[:], xts[k][:, sm * 128:(sm + 1) * 128],
                        wv_sb[k][:], start=(k == 0), stop=(k == DT - 1))
                # evacuate with mask scaling; set+mask ones column
                nc.vector.tensor_scalar_mul(
                    v_sb[:, t, :, 0:DK],
                    ps[:].rearrange("p (h d) -> p h d", h=HLOC),
                    m_sb[:, t:t + 1])
                nc.vector.tensor_scalar_mul(
                    v_sb[:, t, :, DK:DK + 1], ones8[:],
                    m_sb[:, t:t + 1])

            def v_block(n, pool, tag, first=False):
                xts = dma_block(xvT, n, "xv",
                                wtiles=wv_sb if first else None,
                                wsrc=wvT if first else None)
                for sm in range(4):        # s-tiles within block
                    v_group(n, sm, xts, pool, tag)

            # PE warmup: dummy matmuls cover initial DMA latency and start
            # the HAM activity window before the first real matmul.
            dum = wq_pool.tile([128, 512], F32R, tag="dum")
            nc.vector.memset(dum[:].bitcast(F32), 0.0)
            for i in range(_env("K_WARM_MM", 8)):
                pw = psA.tile([128, 512], F32, tag="pa", name=f"warmmm{i}")
                nc.tensor.matmul(pw[:], dum[:, 0:128], dum[:],
                                 start=True, stop=True)
            proj_block(k_tiles, wk_sb, xkT, 0, "xk", wsrc=wkT)
            for n in range(1, NB):
                proj_block(k_tiles, wk_sb, xkT, n, "xk")
            proj_block(q_tiles, wq_sb, xqT, 0, "xq", wsrc=wqT)
            v_block(0, psA, "pa", first=True)
            v_block(1, psA, "pa")
            for k in range(MT):
                nc.sync.dma_start(
                    wo_t[k][:], woT[k * 128:(k + 1) * 128, :].bitcast(F32R))

        # ---------------- Phase B: attention + out-proj ----------------
        SGW = _env("K_SGW", 2)
        with tc.tile_pool(name="ev", bufs=_env("K_EV_BUFS", 3)) as ev, \
             tc.tile_pool(name="x", bufs=2) as xpool, \
             tc.tile_pool(name="small", bufs=_env("K_SMALL_BUFS", 2)) as small, \
             tc.tile_pool(name="o", bufs=2) as opool, \
             tc.tile_pool(name="psS", bufs=_env("K_PSS_BUFS", 3), space="PSUM") as psS, \
             tc.tile_pool(name="psX", bufs=_env("K_XO_BUFS", 2), space="PSUM") as psX:
            x_tiles = [xpool.tile([128, MT, 512], F32R, tag="xs",
                                  name=f"xs{i}") for i in range(2)]
            NSG = KT // SGW

            def outproj_group(oqt, m):
                x_prev = x_tiles[oqt % 2]
                po = psS.tile([128, 512], F32, tag="s", name=f"po{oqt}_{m}")
                for kk in range(MT):
                    nc.tensor.matmul(
                        po[:], wo_t[kk][:, m * 128:(m + 1) * 128],
                        x_prev[:, kk, :], start=(kk == 0), stop=(kk == MT - 1))
                o_sb = opool.tile([128, 512], F32, tag="ob")
                nc.vector.tensor_copy(o_sb[:], po[:])
                (nc.gpsimd if _env("K_OUT_GP", 0) else nc.sync).dma_start(
                    outT[m * 128:(m + 1) * 128, oqt * 512:(oqt + 1) * 512],
                    o_sb[:])

            # side-work: one psS-slot matmul group (or a DMA batch) per sg
            # step. (qt0,p0): v-blocks 2,3 (deadline: attnV eats V tile t at
            # emission slot t//SGW+1). (qt0,p1..3): late q projections n=p.
            # (qt>0,p0): out-projection of qt-1.
            xts_store = {}

            def mk_vdma(nn):
                def f():
                    xts_store[("v", nn)] = dma_block(xvT, nn, "xv")
                return ("dma", f)

            def mk_vg(nn, sm):
                return ("mm", lambda: v_group(nn, sm, xts_store[("v", nn)],
                                              psS, "s"))

            def mk_qdma(nn):
                def f():
                    xts_store[("q", nn)] = dma_block(xqT, nn, "xq")
                return ("dma", f)

            def mk_qg(nn, m):
                return ("mm", lambda: proj_group(q_tiles, wq_sb,
                                                 xts_store[("q", nn)],
                                                 nn, m, psS, "s"))

            side_work = {}
            VOFF = _env("K_VOFF", 0)
            side_work[(0, 0)] = [
                (0, mk_vdma(2)), (max(1, 2 + VOFF), mk_vdma(3)),
                (max(1, 2 + VOFF), mk_vg(2, 0)), (max(2, 3 + VOFF), mk_vg(2, 1)),
                (max(3, 4 + VOFF), mk_vg(2, 2)), (max(4, 5 + VOFF), mk_vg(2, 3)),
                (max(5, 6 + VOFF), mk_vg(3, 0)), (max(6, 7 + VOFF), mk_vg(3, 1)),
                (7 if VOFF < 0 else 99, mk_vg(3, 2)), (99, mk_vg(3, 3)),
            ]
            QOFF = _env("K_QOFF", 3)
            for n in range(1, NB):
                side_work[(0, n)] = [(0, mk_qdma(n))] + [
                    (QOFF + m, mk_qg(n, m)) for m in range(MT)]

            OSPREAD = _env("K_OSPREAD", 4)

            def side_step(qt, p, sg):
                if qt > 0 and p < OSPREAD:
                    per = DT // OSPREAD
                    step = (KT // SGW) // per
                    off = _env("K_OOFF", 1) + (p % 2) * _env("K_OSTAG", 0)
                    if sg % step == off:
                        outproj_group(qt - 1, p * per + sg // step)
                    return
                work = side_work.get((qt, p))
                if not work:
                    return
                did_mm = False
                while work:
                    min_sg, (kind, fn) = work[0]
                    if min_sg > sg or (kind == "mm" and did_mm):
                        break
                    work.pop(0)
                    fn()
                    if kind == "mm":
                        did_mm = True

            def side_flush(qt, p):
                for _, (kind, fn) in side_work.pop((qt, p), []):
                    fn()

            for qt in range(NB):
                x_sb = x_tiles[qt % 2]
                for p in range(MT):        # head pairs; pair p = heads 2p,2p+1
                    heads = (2 * p, 2 * p + 1)
                    ps_x = {h: psX.tile([65, 512], F32, tag="xo",
                                        name=f"psx{qt}_{h}") for h in heads}
                    e_prev = None
                    for sg in range(NSG):
                        ps_s = {h: psS.tile([128, SGW, 512], F32, tag="s",
                                            name=f"pss{qt}_{sg}_{h}")
                                for h in heads}
                        # side work: outproj of qt-1, or late q projection
                        side_step(qt, p, sg)
                        for tt in range(SGW):
                            t = sg * SGW + tt
                            for h in heads:
                                hp = h % 2
                                nc.tensor.matmul(
                                    ps_s[h][:, tt, :],
                                    k_tiles[(p, t // 4)][
                                        hp * 64:(hp + 1) * 64,
                                        (t % 4) * 128:(t % 4 + 1) * 128],
                                    q_tiles[(p, qt)][hp * 64:(hp + 1) * 64, :],
                                    start=True, stop=True)
                        # attnV for the PREVIOUS supergroup (1-sg software lag)
                        if e_prev is not None:
                            psg = sg - 1
                            if _env("K_V_ILV", 0):
                                for tt in range(SGW):
                                    t = psg * SGW + tt
                                    for h in heads:
                                        nc.tensor.matmul(
                                            ps_x[h][:], v_sb[:, t, h, :],
                                            e_prev[h][:, tt, :],
                                            start=(t == 0), stop=(t == KT - 1))
                            else:
                                for h in heads:
                                    for tt in range(SGW):
                                        t = psg * SGW + tt
                                        nc.tensor.matmul(
                                            ps_x[h][:], v_sb[:, t, h, :],
                                            e_prev[h][:, tt, :],
                                            start=(t == 0), stop=(t == KT - 1))
                        e_prev = {}
                        for h in heads:
                            e_sb = ev.tile([128, SGW, 512], F32R, tag="e",
                                           name=f"e{qt}_{sg}_{h}")
                            if _env("K_COPY_EXP", 0):
                                nc.vector.tensor_copy(e_sb[:], ps_s[h][:])
                            else:
                                nc.scalar.activation(e_sb[:], ps_s[h][:], EXP,
                                                     scale=float(SCALE))
                            e_prev[h] = e_sb
                    side_flush(qt, p)
                    last_pair = (qt == NB - 1 and p == MT - 1)
                    for h in heads:            # drain last supergroup + norm
                        psg = NSG - 1
                        for tt in range(SGW):
                            t = psg * SGW + tt
                            nc.tensor.matmul(
                                ps_x[h][:], v_sb[:, t, h, :],
                                e_prev[h][:, tt, :],
                                start=(t == 0), stop=(t == KT - 1))
                        hp = h % 2
                        if last_pair:
                            xr = ps_x[h]   # no next pair: read PSUM directly
                        else:
                            xr = small.tile([65, 512], F32, tag="xr")
                            nc.vector.tensor_copy(xr[:], ps_x[h][:])
                        r = small.tile([1, 512], F32, tag="r",
                                       name=f"r{qt}_{h}")
                        if _env("K_FAST_RECIP", 0):
                            nc.vector.reciprocal_approx_fast(r[:], xr[64:65, :])
                        else:
                            nc.vector.reciprocal(r[:], xr[64:65, :])
                        rb = small.tile([64, 512], F32, tag="rb",
                                        name=f"rb{qt}_{h}")
                        nc.gpsimd.partition_broadcast(rb[:], r[:])
                        meng = nc.gpsimd if _env("K_MUL_GP", 0) else nc.vector
                        if hp == 0:
                            meng.tensor_mul(
                                x_sb[0:64, p, :], xr[0:64, :], rb[:])
                        else:
                            xtmp = small.tile([64, 512], F32R, tag="xr", name=f"xtmp{qt}_{h}")
                            meng.tensor_mul(
                                xtmp[:], xr[0:64, :], rb[:])
                            (nc.gpsimd if _env("K_SHIFT_GP", 0)
                             else nc.sync).dma_start(
                                x_sb[64:128, p, :], xtmp[:])
            for m in range(DT):
                outproj_group(NB - 1, m)
    nc.finalize()
    return nc


def kernel(query, key, value, mask, W_q, W_k, W_v, W_o):
    global _NC
    if _NC is None:
        _NC = _build()
    query = np.asarray(query, dtype=np.float32)
    key = np.asarray(key, dtype=np.float32)
    value = np.asarray(value, dtype=np.float32)
    W_q = np.asarray(W_q, dtype=np.float32)
    W_k = np.asarray(W_k, dtype=np.float32)
    W_v = np.asarray(W_v, dtype=np.float32)
    W_o = np.asarray(W_o, dtype=np.float32)
    mask = np.asarray(mask)

    in_maps = []
    for c in range(NC_CORES):
        b, g = divmod(c, 2)
        hs = slice(g * CW, (g + 1) * CW)
        mrow = (mask[b, 0, 0, :] != 0).astype(np.float32)
        in_maps.append({
            "xqT": np.ascontiguousarray(query[b].T),
            "xkT": np.ascontiguousarray(key[b].T),
            "xvT": np.ascontiguousarray(value[b].T),
            "wqT": np.ascontiguousarray(W_q[hs, :].T),
            "wkT": np.ascontiguousarray(W_k[hs, :].T),
            "wvT": np.ascontiguousarray(W_v[hs, :].T),
            "woT": np.ascontiguousarray(W_o[:, hs].T),
            "maskf": np.ascontiguousarray(mrow.reshape(KT, 128).T),
        })
    res = run_bass_kernel_spmd(_NC, in_maps, core_ids=list(range(NC_CORES)))
    out = np.empty((B, S, DM), np.float32)
    for b in range(B):
        out[b] = (res.results[2 * b]["outT"] + res.results[2 * b + 1]["outT"]).T
    return out



# revision 9
# speedup vs baseline: 1.0010x; 1.0010x over previous
"""MultiHeadAttention Trainium2 kernel.

Sharding: 8 cores = 4 batches x 2 head-groups (8 heads each).
Each core computes, for its (batch b, head-group g):
  Q^T = Wq_g @ Xq^T, K^T = Wk_g @ Xk^T   (bf16 matmuls, [headdim, S] fp16 tiles)
  V   = Xv @ Wv_g^T                       ([S, 512] fp16, +ones col, mask-scaled)
  scores^T[k,q] per head (K=64 fp16 matmuls), e = exp(s/8) -> fp16
  attnV in x-layout: ps_x[q=128, 65] += e_tile[k,q]^T @ [V|1][k,65]
    (full 128x128 PE utilization; one accumulation group per PSUM slot),
  normalize via per-partition reciprocal+tensor_scalar (denominator is the
  65th column), PE-transpose x back to [hd, q], out^T_partial = Wo_g^T.T @ x^T.
Host sums the two head-group partials per batch and transposes back.

attnV runs with a one-PAIR software lag: pair p's attnV groups are emitted
during pair p+1's supergroup slots (one complete 16-matmul accumulation
group per slot), with e tiles of pair p retained in SBUF.

Mask handling: V rows and the ones column are multiplied by mask (0/1), which
masks both the attnV numerator and the softmax denominator exactly.
"""
import contextlib
import os

import numpy as np
import ml_dtypes
import concourse.bass as bass  # noqa: F401
import concourse.tile as tile
from concourse import bacc, mybir
from concourse.bass_utils import run_bass_kernel_spmd

F32 = mybir.dt.float32
BF16 = mybir.dt.bfloat16
F16 = mybir.dt.float16
I16 = mybir.dt.int16
EXP = mybir.ActivationFunctionType.Exp

B, S, DM = 4, 2048, 1024
H = 16
DK = 64
HLOC = 8              # heads per core
CW = HLOC * DK        # 512 local head dims per core
NC_CORES = 8
KT = S // 128         # 16 k-tiles
NB = S // 512         # 4 q/s blocks of 512
MT = CW // 128        # 4 m-tiles of local head dims
DT = DM // 128        # 8 contraction tiles over d_model
SCALE = 1.0 / np.sqrt(DK)
SGW = 2               # k-tiles per supergroup
NSG = KT // SGW       # 8 supergroups

_NC = None


def _env(k, d):
    return int(os.environ.get(k, d))


def _build():
    nc = bacc.Bacc()
    xqT = nc.dram_tensor("xqT", [DM, S], BF16, kind="ExternalInput")
    xkT = nc.dram_tensor("xkT", [DM, S], BF16, kind="ExternalInput")
    xvT = nc.dram_tensor("xvT", [DM, S], BF16, kind="ExternalInput")
    wqT = nc.dram_tensor("wqT", [DM, CW], BF16, kind="ExternalInput")
    wkT = nc.dram_tensor("wkT", [DM, CW], BF16, kind="ExternalInput")
    wvT = nc.dram_tensor("wvT", [DM, CW], BF16, kind="ExternalInput")
    woT = nc.dram_tensor("woT", [CW, DM], F16, kind="ExternalInput")
    maskf = nc.dram_tensor("maskf", [128, KT], F32, kind="ExternalInput")
    ident = nc.dram_tensor("ident", [128, 128], F16, kind="ExternalInput")
    outT = nc.dram_tensor("outT", [DM, S], F32, kind="ExternalOutput")
    xdbg = (nc.dram_tensor("xdbg", [NB, 128, MT, 512], F16,
                           kind="ExternalOutput")
            if _env("K_DEBUG_X", 0) else None)

    # DVE fast-exp (int16 bit trick) constants: i16 = s*c1 + c2 bitcast f16
    FE_C1 = float(SCALE * np.log2(np.e) * 1024.0)
    FE_C2 = float(15.0 * 1024.0 - 486411.0 / 8192.0 + _env("K_FE_HALF", 0) * 0.5)
    EXPDVE_MOD = _env("K_EXPDVE_MOD", 0)   # offload exp of sg%MOD==MOD-1 to DVE

    with tile.TileContext(nc) as tc, contextlib.ExitStack() as ctx:
        persist = ctx.enter_context(tc.tile_pool(name="persist", bufs=1))

        # --- persistent tiles: mask, identity, Q^T/K^T slices, V ---
        m_sb = persist.tile([128, KT], F32)
        nc.sync.dma_start(m_sb[:], maskf[:])
        id16 = persist.tile([128, 128], F16, tag="id")
        nc.sync.dma_start(id16[:], ident[:])
        ones8 = persist.tile([128, HLOC], F32)
        nc.vector.memset(ones8[:], 1.0)
        warm = persist.tile([1, 1], F32)
        nc.scalar.activation(warm[:], ones8[0:1, 0:1], EXP, scale=1.0)
        q_tiles = {}   # (m, nb) -> [128, 512] f16  (Q^T slice)
        k_tiles = {}
        for m in range(MT):
            for n in range(NB):
                q_tiles[(m, n)] = persist.tile(
                    [128, 512], F16, tag=f"q{m}_{n}", name=f"q{m}_{n}")
                k_tiles[(m, n)] = persist.tile(
                    [128, 512], F16, tag=f"k{m}_{n}", name=f"k{m}_{n}")
        v_sb = persist.tile([128, KT, HLOC, DK + 1], F16, tag="v")

        # ---------------- Phase A: projections ----------------
        wq_pool = ctx.enter_context(tc.tile_pool(name="wqp", bufs=1))
        xt = ctx.enter_context(tc.tile_pool(name="xt", bufs=_env("K_XT_BUFS", 8)))
        ctxA = contextlib.ExitStack()
        with ctxA:
            wkv_pool = ctxA.enter_context(tc.tile_pool(name="wkv", bufs=1))
            psA = ctxA.enter_context(tc.tile_pool(name="psA", bufs=8, space="PSUM"))
            wq_sb = [wq_pool.tile([128, CW], BF16, tag=f"wq{k}", name=f"wq{k}")
                     for k in range(DT)]
            wk_sb = [wkv_pool.tile([128, CW], BF16, tag=f"wk{k}", name=f"wk{k}")
                     for k in range(DT)]
            wv_sb = [wq_pool.tile([128, CW], BF16, tag=f"wv{k}", name=f"wv{k}")
                     for k in range(DT)]
            wo_t = [persist.tile([128, DM], F16, tag=f"wo{k}", name=f"wo{k}")
                    for k in range(MT)]

            def w_dma(tiles, src, k):
                nc.sync.dma_start(tiles[k][:], src[k * 128:(k + 1) * 128, :])

            def dma_block(src, n, nm, wtiles=None, wsrc=None):
                tiles = [xt.tile([128, 512], BF16, tag="xt",
                                 name=f"{nm}{n}_{i}") for i in range(DT)]
                for k in range(DT):
                    if wtiles is not None:
                        w_dma(wtiles, wsrc, k)
                    nc.sync.dma_start(
                        tiles[k][:],
                        src[k * 128:(k + 1) * 128, n * 512:(n + 1) * 512])
                return tiles

            def proj_group(dst_tiles, w_sb, xts, n, m, pool, tag):
                ps = pool.tile([128, 512], F32, tag=tag, name=f"pj{n}_{m}_{tag}")
                for k in range(DT):
                    nc.tensor.matmul(
                        ps[:], w_sb[k][:, m * 128:(m + 1) * 128],
                        xts[k][:], start=(k == 0), stop=(k == DT - 1))
                nc.vector.tensor_copy(dst_tiles[(m, n)][:], ps[:])

            def proj_block(dst_tiles, w_sb, src, n, nm, wsrc=None):
                xts = dma_block(src, n, nm,
                                wtiles=w_sb if wsrc is not None else None,
                                wsrc=wsrc)
                for m in range(MT):
                    proj_group(dst_tiles, w_sb, xts, n, m, psA, "pa")

            def v_group(n, sm, xts, pool, tag):
                t = n * 4 + sm
                ps = pool.tile([128, 512], F32, tag=tag, name=f"vps{n}_{sm}")
                for k in range(DT):
                    nc.tensor.matmul(
                        ps[:], xts[k][:, sm * 128:(sm + 1) * 128],
                        wv_sb[k][:], start=(k == 0), stop=(k == DT - 1))
                # evacuate with mask scaling; set+mask ones column
                nc.vector.tensor_scalar_mul(
                    v_sb[:, t, :, 0:DK],
                    ps[:].rearrange("p (h d) -> p h d", h=HLOC),
                    m_sb[:, t:t + 1])
                nc.vector.tensor_scalar_mul(
                    v_sb[:, t, :, DK:DK + 1], ones8[:],
                    m_sb[:, t:t + 1])

            def v_block(n, pool, tag, first=False):
                xts = dma_block(xvT, n, "xv",
                                wtiles=wv_sb if first else None,
                                wsrc=wvT if first else None)
                for sm in range(4):        # s-tiles within block
                    v_group(n, sm, xts, pool, tag)

            # PE warmup: dummy matmuls cover initial DMA latency and start
            # the HAM activity window before the first real matmul.
            dum = wq_pool.tile([128, 512], BF16, tag="dum")
            nc.vector.memset(dum[:], 0.0)
            for i in range(_env("K_WARM_MM", 8)):
                pw = psA.tile([128, 512], F32, tag="pa", name=f"warmmm{i}")
                nc.tensor.matmul(pw[:], dum[:, 0:128], dum[:],
                                 start=True, stop=True)
            proj_block(k_tiles, wk_sb, xkT, 0, "xk", wsrc=wkT)
            for n in range(1, NB):
                proj_block(k_tiles, wk_sb, xkT, n, "xk")
            proj_block(q_tiles, wq_sb, xqT, 0, "xq", wsrc=wqT)
            v_block(0, psA, "pa", first=True)
            v_block(1, psA, "pa")
            for k in range(MT):
                nc.sync.dma_start(
                    wo_t[k][:], woT[k * 128:(k + 1) * 128, :])

        # ---------------- Phase B: attention + out-proj ----------------
        with tc.tile_pool(name="ev", bufs=_env("K_EV_BUFS", 36)) as ev, \
             tc.tile_pool(name="x", bufs=2) as xpool, \
             tc.tile_pool(name="small", bufs=_env("K_SMALL_BUFS", 4)) as small, \
             tc.tile_pool(name="o", bufs=2) as opool, \
             tc.tile_pool(name="psS", bufs=_env("K_PSS_BUFS", 3), space="PSUM") as psS, \
             tc.tile_pool(name="psX", bufs=1, space="PSUM") as psX:
            # one persistent PSUM accumulator; slice g = group (qc, hp).
            # Padded to a 128-f32 group stride so no slice straddles a
            # 2KB PSUM bank boundary (matmul outs must stay in one bank).
            px_all = psX.tile([128, 2 * MT, DK + 1], F32, tag="xo",
                              padded_shape=[128, 2 * MT, 128])
            # x in q-partition layout, per qt: 4 tiles [128 q, 512 hd] f16
            x_sb = [[xpool.tile([128, 512], F16, tag=f"xs{qc}",
                                name=f"xs{i}_{qc}") for qc in range(4)]
                    for i in range(2)]
            # x^T tiles for outproj, per qt: [128 hd, MT, 512 q] f16
            xT_tiles = [xpool.tile([128, MT, 512], F16, tag="xT",
                                   name=f"xT{i}") for i in range(2)]

            def outproj_group(oqt, m):
                xT_prev = xT_tiles[oqt % 2]
                po = psS.tile([128, 512], F32, tag="s", name=f"po{oqt}_{m}")
                for kk in range(MT):
                    nc.tensor.matmul(
                        po[:], wo_t[kk][:, m * 128:(m + 1) * 128],
                        xT_prev[:, kk, :], start=(kk == 0), stop=(kk == MT - 1))
                o_sb = opool.tile([128, 512], F32, tag="ob")
                nc.vector.tensor_copy(o_sb[:], po[:])
                nc.sync.dma_start(
                    outT[m * 128:(m + 1) * 128, oqt * 512:(oqt + 1) * 512],
                    o_sb[:])

            # side-work: one psS-slot matmul group (or a DMA batch) per sg
            # step. (qt0,p0): v-blocks 2,3. (qt0,p1..3): late q projections.
            # (qt>0,p): out-projection of qt-1.
            xts_store = {}

            def mk_vdma(nn):
                def f():
                    xts_store[("v", nn)] = dma_block(xvT, nn, "xv")
                return ("dma", f)

            def mk_vg(nn, sm):
                return ("mm", lambda: v_group(nn, sm, xts_store[("v", nn)],
                                              psS, "s"))

            def mk_qdma(nn):
                def f():
                    xts_store[("q", nn)] = dma_block(xqT, nn, "xq")
                return ("dma", f)

            def mk_qg(nn, m):
                return ("mm", lambda: proj_group(q_tiles, wq_sb,
                                                 xts_store[("q", nn)],
                                                 nn, m, psS, "s"))

            side_work = {}
            side_work[(0, 0)] = [
                (0, mk_vdma(2)), (1, mk_vdma(3)),
                (1, mk_vg(2, 0)), (2, mk_vg(2, 1)),
                (3, mk_vg(2, 2)), (4, mk_vg(2, 3)),
                (5, mk_vg(3, 0)), (6, mk_vg(3, 1)),
                (7, mk_vg(3, 2)), (99, mk_vg(3, 3)),
            ]
            QOFF = _env("K_QOFF", 3)
            for n in range(1, NB):
                side_work[(0, n)] = [(0, mk_qdma(n))] + [
                    (QOFF + m, mk_qg(n, m)) for m in range(MT)]

            # out-projection of qt-1 spread over pairs p=1..3 of qt (the
            # last transposes of (qt-1, 3) are only emitted during (qt, 0)).
            OPROJ_M = {1: (0, 1, 2), 2: (3, 4, 5), 3: (6, 7)}

            def side_step(qt, p, sg):
                if qt > 0 and p >= 1:
                    ms = OPROJ_M[p]
                    off = _env("K_OOFF", 1)
                    if sg >= off and (sg - off) % 2 == 0:
                        idx = (sg - off) // 2
                        if idx < len(ms):
                            outproj_group(qt - 1, ms[idx])
                    return
                work = side_work.get((qt, p))
                if not work:
                    return
                did_mm = False
                while work:
                    min_sg, (kind, fn) = work[0]
                    if min_sg > sg or (kind == "mm" and did_mm):
                        break
                    work.pop(0)
                    fn()
                    if kind == "mm":
                        did_mm = True

            def side_flush(qt, p):
                for _, (kind, fn) in side_work.pop((qt, p), []):
                    fn()

            # Pair pipeline: each slot of pair (qt,p) also runs one complete
            # attnV accumulation group of the PREVIOUS pair, plus its norm.
            # prev_pair state: (qt, p, e_tiles{h: [tile per sg]}).
            prev_pair = None

            def attnv_group(pqt, pp, e_tiles, g):
                """Emit accumulation group g (=(qc, hp)) of pair pp: 16
                matmuls into one psX slot, then normalize into x_sb."""
                qc, hp = divmod(g, 2)
                h = 2 * pp + hp
                px = px_all[:, g, :]
                for t in range(KT):
                    nc.tensor.matmul(
                        px,
                        e_tiles[hp][t // SGW][:, t % SGW,
                                              qc * 128:(qc + 1) * 128],
                        v_sb[:, t, h, :],
                        start=(t == 0), stop=(t == KT - 1))
                r = small.tile([128, 1], F32, tag="r",
                               name=f"r{pqt}_{pp}_{g}")
                nc.vector.reciprocal(r[:], px[:, DK:DK + 1])
                nc.vector.tensor_scalar(
                    x_sb[pqt % 2][qc][:, pp * 128 + hp * 64:
                                      pp * 128 + (hp + 1) * 64],
                    px[:, 0:DK], r[:], None, mybir.AluOpType.mult)

            def transpose_block(pqt, pp, qc):
                """x_sb[qc][:, pp*128:+128] -> xT[:, pp, qc*128:+128]."""
                pt = psS.tile([128, 128], F16, tag="s",
                              name=f"tr{pqt}_{pp}_{qc}")
                nc.tensor.transpose(
                    pt[:], x_sb[pqt % 2][qc][:, pp * 128:(pp + 1) * 128],
                    id16[:])
                nc.vector.tensor_copy(
                    xT_tiles[pqt % 2][:, pp, qc * 128:(qc + 1) * 128], pt[:])
                if xdbg is not None:
                    nc.sync.dma_start(
                        xdbg[pqt, :, pp, qc * 128:(qc + 1) * 128],
                        xT_tiles[pqt % 2][:, pp, qc * 128:(qc + 1) * 128])

            def drain_pair(pair, sg_from):
                """Emit remaining attnV groups + transposes of `pair`."""
                if pair is None:
                    return
                pqt, pp, e_tiles = pair
                for g in range(sg_from, NSG):
                    attnv_group(pqt, pp, e_tiles, g)
                for qc in range(4):
                    transpose_block(pqt, pp, qc)

            pair_list = [(qt, p) for qt in range(NB) for p in range(MT)]
            for qt, p in pair_list:
                heads = (2 * p, 2 * p + 1)
                e_tiles = {0: [None] * NSG, 1: [None] * NSG}
                for sg in range(NSG):
                    ps_s = {h: psS.tile([128, SGW, 512], F32, tag="s",
                                        name=f"pss{qt}_{sg}_{h}")
                            for h in heads}
                    side_step(qt, p, sg)
                    for tt in range(SGW):
                        t = sg * SGW + tt
                        for h in heads:
                            hp = h % 2
                            nc.tensor.matmul(
                                ps_s[h][:, tt, :],
                                k_tiles[(p, t // 4)][
                                    hp * 64:(hp + 1) * 64,
                                    (t % 4) * 128:(t % 4 + 1) * 128],
                                q_tiles[(p, qt)][hp * 64:(hp + 1) * 64, :],
                                start=True, stop=True)
                    # one attnV group of the previous pair per slot
                    if prev_pair is not None:
                        attnv_group(prev_pair[0], prev_pair[1],
                                    prev_pair[2], sg)
                        # transposes of the previous pair trail the groups
                        if sg >= 2 and sg % 2 == 0:
                            transpose_block(prev_pair[0], prev_pair[1],
                                            sg // 2 - 1)
                        elif sg == NSG - 1:
                            transpose_block(prev_pair[0], prev_pair[1], 3)
                    for h in heads:
                        hp = h % 2
                        e_sb = ev.tile([128, SGW, 512], F16, tag="e",
                                       name=f"e{qt}_{p}_{sg}_{h}")
                        if EXPDVE_MOD and sg % EXPDVE_MOD == EXPDVE_MOD - 1:
                            nc.vector.tensor_scalar(
                                e_sb[:].bitcast(I16), ps_s[h][:],
                                FE_C1, FE_C2,
                                mybir.AluOpType.mult, mybir.AluOpType.add)
                        else:
                            nc.scalar.activation(e_sb[:], ps_s[h][:], EXP,
                                                 scale=float(SCALE))
                        e_tiles[hp][sg] = e_sb
                side_flush(qt, p)
                prev_pair = (qt, p, e_tiles)
            # tail: drain the last pair and emit the final out-projection
            drain_pair(prev_pair, 0)
            for m in range(DT):
                outproj_group(NB - 1, m)
    nc.finalize()
    return nc


def kernel(query, key, value, mask, W_q, W_k, W_v, W_o):
    global _NC
    if _NC is None:
        _NC = _build()
    BF = ml_dtypes.bfloat16
    query = np.asarray(query, dtype=np.float32)
    key = np.asarray(key, dtype=np.float32)
    value = np.asarray(value, dtype=np.float32)
    W_q = np.asarray(W_q, dtype=np.float32)
    W_k = np.asarray(W_k, dtype=np.float32)
    W_v = np.asarray(W_v, dtype=np.float32)
    W_o = np.asarray(W_o, dtype=np.float32)
    mask = np.asarray(mask)
    ident = np.eye(128, dtype=np.float16)

    in_maps = []
    for c in range(NC_CORES):
        b, g = divmod(c, 2)
        hs = slice(g * CW, (g + 1) * CW)
        mrow = (mask[b, 0, 0, :] != 0).astype(np.float32)
        in_maps.append({
            "xqT": np.ascontiguousarray(query[b].T).astype(BF),
            "xkT": np.ascontiguousarray(key[b].T).astype(BF),
            "xvT": np.ascontiguousarray(value[b].T).astype(BF),
            "wqT": np.ascontiguousarray(W_q[hs, :].T).astype(BF),
            "wkT": np.ascontiguousarray(W_k[hs, :].T).astype(BF),
            "wvT": np.ascontiguousarray(W_v[hs, :].T).astype(BF),
            "woT": np.ascontiguousarray(W_o[:, hs].T).astype(np.float16),
            "maskf": np.ascontiguousarray(mrow.reshape(KT, 128).T),
            "ident": ident,
        })
    res = run_bass_kernel_spmd(_NC, in_maps, core_ids=list(range(NC_CORES)))
    out = np.empty((B, S, DM), np.float32)
    for b in range(B):
        out[b] = (res.results[2 * b]["outT"] + res.results[2 * b + 1]["outT"]).T
    return out


# revision 15
# speedup vs baseline: 1.0179x; 1.0168x over previous
"""MultiHeadAttention Trainium2 kernel.

Sharding: 8 cores = 4 batches x 2 head-groups (8 heads each).
Each core computes, for its (batch b, head-group g):
  Q^T = Wq_g @ Xq^T, K^T = Wk_g @ Xk^T   (bf16 matmuls, [headdim, S] fp16 tiles)
  V   = Xv @ Wv_g^T                       ([S, 512] fp16, +ones col, mask-scaled)
  scores^T[k,q] per head (K=64 fp16 matmuls), e = exp(s/8) -> fp16
  attnV in x-layout: ps_x[q=128, 65] += e_tile[k,q]^T @ [V|1][k,65]
    (full 128x128 PE utilization; one accumulation group per PSUM slot),
  normalize via per-partition reciprocal+tensor_scalar (denominator is the
  65th column), PE-transpose x back to [hd, q], out^T_partial = Wo_g^T.T @ x^T.
Host sums the two head-group partials per batch and transposes back.

attnV runs with a one-PAIR software lag: pair p's attnV groups are emitted
during pair p+1's supergroup slots (one complete 16-matmul accumulation
group per slot), with e tiles of pair p retained in SBUF.

Mask handling: V rows and the ones column are multiplied by mask (0/1), which
masks both the attnV numerator and the softmax denominator exactly.
"""
import contextlib
import os

import numpy as np
import ml_dtypes
import concourse.bass as bass  # noqa: F401
import concourse.tile as tile
from concourse import bacc, mybir
from concourse.bass_utils import run_bass_kernel_spmd

F32 = mybir.dt.float32
BF16 = mybir.dt.bfloat16
F16 = mybir.dt.float16
I16 = mybir.dt.int16
EXP = mybir.ActivationFunctionType.Exp

B, S, DM = 4, 2048, 1024
H = 16
DK = 64
HLOC = 8              # heads per core
CW = HLOC * DK        # 512 local head dims per core
NC_CORES = 8
KT = S // 128         # 16 k-tiles
NB = S // 512         # 4 q/s blocks of 512
MT = CW // 128        # 4 m-tiles of local head dims
DT = DM // 128        # 8 contraction tiles over d_model
SCALE = 1.0 / np.sqrt(DK)
SGW = 2               # k-tiles per supergroup
NSG = KT // SGW       # 8 supergroups

_NC = None


def _env(k, d):
    return int(os.environ.get(k, d))


def _build():
    nc = bacc.Bacc()
    xqT = nc.dram_tensor("xqT", [DM, S], BF16, kind="ExternalInput")
    xkT = nc.dram_tensor("xkT", [DM, S], BF16, kind="ExternalInput")
    xvT = nc.dram_tensor("xvT", [DM, S], BF16, kind="ExternalInput")
    wqT = nc.dram_tensor("wqT", [DM, CW], BF16, kind="ExternalInput")
    wkT = nc.dram_tensor("wkT", [DM, CW], BF16, kind="ExternalInput")
    wvT = nc.dram_tensor("wvT", [DM, CW], BF16, kind="ExternalInput")
    woT = nc.dram_tensor("woT", [CW, DM], F16, kind="ExternalInput")
    maskf = nc.dram_tensor("maskf", [128, KT], F32, kind="ExternalInput")
    ident = nc.dram_tensor("ident", [128, 128], F16, kind="ExternalInput")
    outT = nc.dram_tensor("outT", [DM, S], F32, kind="ExternalOutput")
    xdbg = (nc.dram_tensor("xdbg", [NB, 128, MT, 512], F16,
                           kind="ExternalOutput")
            if _env("K_DEBUG_X", 0) else None)

    # DVE fast-exp (int16 bit trick) constants: i16 = s*c1 + c2 bitcast f16
    FE_C1 = float(SCALE * np.log2(np.e) * 1024.0)
    FE_C2 = float(15.0 * 1024.0 - 486411.0 / 8192.0 + _env("K_FE_HALF", 0) * 0.5)
    EXPDVE_MOD = _env("K_EXPDVE_MOD", 0)   # offload exp of sg%MOD==MOD-1 to DVE

    with tile.TileContext(nc) as tc, contextlib.ExitStack() as ctx:
        persist = ctx.enter_context(tc.tile_pool(name="persist", bufs=1))

        # --- persistent tiles: mask, identity, Q^T/K^T slices, V ---
        m_sb = persist.tile([128, KT], F32)
        nc.sync.dma_start(m_sb[:], maskf[:])
        id16 = persist.tile([128, 128], F16, tag="id")
        nc.sync.dma_start(id16[:], ident[:])
        ones8 = persist.tile([128, HLOC], F32)
        nc.vector.memset(ones8[:], 1.0)
        warm = persist.tile([1, 1], F32)
        nc.scalar.activation(warm[:], ones8[0:1, 0:1], EXP, scale=1.0)
        q_tiles = {}   # (m, nb) -> [128, 512] f16  (Q^T slice)
        k_tiles = {}
        for m in range(MT):
            for n in range(NB):
                q_tiles[(m, n)] = persist.tile(
                    [128, 512], F16, tag=f"q{m}_{n}", name=f"q{m}_{n}")
                k_tiles[(m, n)] = persist.tile(
                    [128, 512], F16, tag=f"k{m}_{n}", name=f"k{m}_{n}")
        v_sb = persist.tile([128, KT, HLOC, DK + 1], F16, tag="v")

        # ---------------- Phase A: projections ----------------
        wq_pool = ctx.enter_context(tc.tile_pool(name="wqp", bufs=1))
        xt = ctx.enter_context(tc.tile_pool(name="xt", bufs=_env("K_XT_BUFS", 3)))
        ctxA = contextlib.ExitStack()
        with ctxA:
            wkv_pool = ctxA.enter_context(tc.tile_pool(name="wkv", bufs=1))
            psA = ctxA.enter_context(tc.tile_pool(name="psA", bufs=8, space="PSUM"))
            wq_t = wq_pool.tile([128, DT, CW], BF16, tag="wq")
            wk_t = wkv_pool.tile([128, DT, CW], BF16, tag="wk")
            wv_t = wq_pool.tile([128, DT, CW], BF16, tag="wv")
            wq_sb = [wq_t[:, k, :] for k in range(DT)]
            wk_sb = [wk_t[:, k, :] for k in range(DT)]
            wv_sb = [wv_t[:, k, :] for k in range(DT)]
            wo_all = persist.tile([128, MT, DM], F16, tag="wo")
            wo_t = [wo_all[:, k, :] for k in range(MT)]

            def dma_block(src, n, nm, wtiles=None, wsrc=None):
                blk = xt.tile([128, DT, 512], BF16, tag="xt",
                              name=f"{nm}{n}")
                if wtiles is not None:
                    nc.sync.dma_start(
                        wtiles,
                        wsrc[:, :].rearrange("(k p) c -> p k c", p=128))
                nc.sync.dma_start(
                    blk[:],
                    src[:, n * 512:(n + 1) * 512].rearrange(
                        "(k p) c -> p k c", p=128))
                return [blk[:, k, :] for k in range(DT)]

            def proj_group(dst_tiles, w_sb, xts, n, m, pool, tag):
                ps = pool.tile([128, 512], F32, tag=tag, name=f"pj{n}_{m}_{tag}")
                for k in range(DT):
                    nc.tensor.matmul(
                        ps[:], w_sb[k][:, m * 128:(m + 1) * 128],
                        xts[k][:], start=(k == 0), stop=(k == DT - 1))
                nc.vector.tensor_copy(dst_tiles[(m, n)][:], ps[:])

            def proj_block(dst_tiles, w_sb, wt, src, n, nm, wsrc=None):
                xts = dma_block(src, n, nm,
                                wtiles=wt if wsrc is not None else None,
                                wsrc=wsrc)
                for m in range(MT):
                    proj_group(dst_tiles, w_sb, xts, n, m, psA, "pa")

            def v_group(n, sm, xts, pool, tag):
                t = n * 4 + sm
                ps = pool.tile([128, 512], F32, tag=tag, name=f"vps{n}_{sm}")
                for k in range(DT):
                    nc.tensor.matmul(
                        ps[:], xts[k][:, sm * 128:(sm + 1) * 128],
                        wv_sb[k][:], start=(k == 0), stop=(k == DT - 1))
                # evacuate with mask scaling; set+mask ones column
                nc.vector.tensor_scalar_mul(
                    v_sb[:, t, :, 0:DK],
                    ps[:].rearrange("p (h d) -> p h d", h=HLOC),
                    m_sb[:, t:t + 1])
                nc.vector.tensor_scalar_mul(
                    v_sb[:, t, :, DK:DK + 1], ones8[:],
                    m_sb[:, t:t + 1])

            def v_block(n, pool, tag, first=False):
                xts = dma_block(xvT, n, "xv",
                                wtiles=wv_t[:] if first else None,
                                wsrc=wvT if first else None)
                for sm in range(4):        # s-tiles within block
                    v_group(n, sm, xts, pool, tag)

            # PE warmup: dummy matmuls cover initial DMA latency and start
            # the HAM activity window before the first real matmul.
            dum = wq_pool.tile([128, 512], BF16, tag="dum")
            nc.vector.memset(dum[:], 0.0)
            for i in range(_env("K_WARM_MM", 8)):
                pw = psA.tile([128, 512], F32, tag="pa", name=f"warmmm{i}")
                nc.tensor.matmul(pw[:], dum[:, 0:128], dum[:],
                                 start=True, stop=True)
            proj_block(k_tiles, wk_sb, wk_t[:], xkT, 0, "xk", wsrc=wkT)
            for n in range(1, NB):
                proj_block(k_tiles, wk_sb, wk_t[:], xkT, n, "xk")
            proj_block(q_tiles, wq_sb, wq_t[:], xqT, 0, "xq", wsrc=wqT)
            v_block(0, psA, "pa", first=True)
            v_block(1, psA, "pa")
            nc.sync.dma_start(
                wo_all[:], woT[:, :].rearrange("(k p) c -> p k c", p=128))

        # ---------------- Phase B: attention + out-proj ----------------
        with tc.tile_pool(name="ev", bufs=_env("K_EV_BUFS", 36)) as ev, \
             tc.tile_pool(name="x", bufs=2) as xpool, \
             tc.tile_pool(name="small", bufs=_env("K_SMALL_BUFS", 4)) as small, \
             tc.tile_pool(name="o", bufs=2) as opool, \
             tc.tile_pool(name="psS", bufs=_env("K_PSS_BUFS", 3), space="PSUM") as psS, \
             tc.tile_pool(name="psX", bufs=1, space="PSUM") as psX:
            # one persistent PSUM accumulator; slice g = group (qc, hp).
            # Padded to a 128-f32 group stride so no slice straddles a
            # 2KB PSUM bank boundary (matmul outs must stay in one bank).
            px_all = psX.tile([128, 2 * MT, DK + 1], F32, tag="xo",
                              padded_shape=[128, 2 * MT, 128])
            # x in q-partition layout, per qt: 4 tiles [128 q, 512 hd] f16
            x_sb = [[xpool.tile([128, 512], F16, tag=f"xs{qc}",
                                name=f"xs{i}_{qc}") for qc in range(4)]
                    for i in range(2)]
            # x^T tiles for outproj, per qt: [128 hd, MT, 512 q] f16
            xT_tiles = [xpool.tile([128, MT, 512], F16, tag="xT",
                                   name=f"xT{i}") for i in range(2)]

            def outproj_group(oqt, m):
                xT_prev = xT_tiles[oqt % 2]
                po = psS.tile([128, 512], F32, tag="s", name=f"po{oqt}_{m}")
                for kk in range(MT):
                    nc.tensor.matmul(
                        po[:], wo_t[kk][:, m * 128:(m + 1) * 128],
                        xT_prev[:, kk, :], start=(kk == 0), stop=(kk == MT - 1))
                o_sb = opool.tile([128, 512], F32, tag="ob")
                nc.vector.tensor_copy(o_sb[:], po[:])
                nc.sync.dma_start(
                    outT[m * 128:(m + 1) * 128, oqt * 512:(oqt + 1) * 512],
                    o_sb[:])

            # side-work: one psS-slot matmul group (or a DMA batch) per sg
            # step. (qt0,p0): v-blocks 2,3. (qt0,p1..3): late q projections.
            # (qt>0,p): out-projection of qt-1.
            xts_store = {}

            def mk_vdma(nn):
                def f():
                    xts_store[("v", nn)] = dma_block(xvT, nn, "xv")
                return ("dma", f)

            def mk_vg(nn, sm):
                return ("mm", lambda: v_group(nn, sm, xts_store[("v", nn)],
                                              psS, "s"))

            def mk_qdma(nn):
                def f():
                    xts_store[("q", nn)] = dma_block(xqT, nn, "xq")
                return ("dma", f)

            def mk_qg(nn, m):
                return ("mm", lambda: proj_group(q_tiles, wq_sb,
                                                 xts_store[("q", nn)],
                                                 nn, m, psS, "s"))

            side_work = {}
            side_work[(0, 0)] = [
                (0, mk_vdma(2)), (1, mk_vdma(3)),
                (1, mk_vg(2, 0)), (2, mk_vg(2, 1)),
                (3, mk_vg(2, 2)), (4, mk_vg(2, 3)),
                (5, mk_vg(3, 0)), (6, mk_vg(3, 1)),
                (7, mk_vg(3, 2)), (99, mk_vg(3, 3)),
            ]
            QOFF = _env("K_QOFF", 3)
            for n in range(1, NB):
                side_work[(0, n)] = [(0, mk_qdma(n))] + [
                    (QOFF + m, mk_qg(n, m)) for m in range(MT)]

            # out-projection of qt-1 spread over pairs p=1..3 of qt (the
            # last transposes of (qt-1, 3) are only emitted during (qt, 0)).
            OPROJ_M = {1: (0, 1, 2), 2: (3, 4, 5), 3: (6, 7)}

            def side_step(qt, p, sg):
                if qt > 0 and p >= 1:
                    ms = OPROJ_M[p]
                    off = _env("K_OOFF", 1)
                    if sg >= off and (sg - off) % 2 == 0:
                        idx = (sg - off) // 2
                        if idx < len(ms):
                            outproj_group(qt - 1, ms[idx])
                    return
                work = side_work.get((qt, p))
                if not work:
                    return
                did_mm = False
                while work:
                    min_sg, (kind, fn) = work[0]
                    if min_sg > sg or (kind == "mm" and did_mm):
                        break
                    work.pop(0)
                    fn()
                    if kind == "mm":
                        did_mm = True

            def side_flush(qt, p):
                for _, (kind, fn) in side_work.pop((qt, p), []):
                    fn()

            # Pair pipeline: each slot of pair (qt,p) also runs one complete
            # attnV accumulation group of the PREVIOUS pair, plus its norm.
            # prev_pair state: (qt, p, e_tiles{h: [tile per sg]}).
            prev_pair = None

            def attnv_group(pqt, pp, e_tiles, g):
                """Emit accumulation group g (=(qc, hp)) of pair pp: 16
                matmuls into one psX slot, then normalize into x_sb."""
                qc, hp = divmod(g, 2)
                h = 2 * pp + hp
                px = px_all[:, g, :]
                for t in range(KT):
                    nc.tensor.matmul(
                        px,
                        e_tiles[hp][t // SGW][:, t % SGW,
                                              qc * 128:(qc + 1) * 128],
                        v_sb[:, t, h, :],
                        start=(t == 0), stop=(t == KT - 1))
                r = small.tile([128, 1], F32, tag="r",
                               name=f"r{pqt}_{pp}_{g}")
                nc.vector.reciprocal(r[:], px[:, DK:DK + 1])
                nc.vector.tensor_scalar(
                    x_sb[pqt % 2][qc][:, pp * 128 + hp * 64:
                                      pp * 128 + (hp + 1) * 64],
                    px[:, 0:DK], r[:], None, mybir.AluOpType.mult)

            def transpose_block(pqt, pp, qc):
                """x_sb[qc][:, pp*128:+128] -> xT[:, pp, qc*128:+128]."""
                pt = psS.tile([128, 128], F16, tag="s",
                              name=f"tr{pqt}_{pp}_{qc}")
                nc.tensor.transpose(
                    pt[:], x_sb[pqt % 2][qc][:, pp * 128:(pp + 1) * 128],
                    id16[:])
                nc.vector.tensor_copy(
                    xT_tiles[pqt % 2][:, pp, qc * 128:(qc + 1) * 128], pt[:])
                if xdbg is not None:
                    nc.sync.dma_start(
                        xdbg[pqt, :, pp, qc * 128:(qc + 1) * 128],
                        xT_tiles[pqt % 2][:, pp, qc * 128:(qc + 1) * 128])

            def drain_pair(pair, sg_from):
                """Emit remaining attnV groups + transposes of `pair`."""
                if pair is None:
                    return
                pqt, pp, e_tiles = pair
                for g in range(sg_from, NSG):
                    attnv_group(pqt, pp, e_tiles, g)
                for qc in range(4):
                    transpose_block(pqt, pp, qc)

            pair_list = [(qt, p) for qt in range(NB) for p in range(MT)]
            for qt, p in pair_list:
                heads = (2 * p, 2 * p + 1)
                e_tiles = {0: [None] * NSG, 1: [None] * NSG}
                for sg in range(NSG):
                    ps_s = {h: psS.tile([128, SGW, 512], F32, tag="s",
                                        name=f"pss{qt}_{sg}_{h}")
                            for h in heads}
                    side_step(qt, p, sg)
                    for tt in range(SGW):
                        t = sg * SGW + tt
                        for h in heads:
                            hp = h % 2
                            nc.tensor.matmul(
                                ps_s[h][:, tt, :],
                                k_tiles[(p, t // 4)][
                                    hp * 64:(hp + 1) * 64,
                                    (t % 4) * 128:(t % 4 + 1) * 128],
                                q_tiles[(p, qt)][hp * 64:(hp + 1) * 64, :],
                                start=True, stop=True)
                    # one attnV group of the previous pair per slot
                    if prev_pair is not None:
                        attnv_group(prev_pair[0], prev_pair[1],
                                    prev_pair[2], sg)
                        # transposes of the previous pair trail the groups
                        if sg >= 2 and sg % 2 == 0:
                            transpose_block(prev_pair[0], prev_pair[1],
                                            sg // 2 - 1)
                        elif sg == NSG - 1:
                            transpose_block(prev_pair[0], prev_pair[1], 3)
                    for h in heads:
                        hp = h % 2
                        e_sb = ev.tile([128, SGW, 512], F16, tag="e",
                                       name=f"e{qt}_{p}_{sg}_{h}")
                        if EXPDVE_MOD and sg % EXPDVE_MOD == EXPDVE_MOD - 1:
                            nc.vector.tensor_scalar(
                                e_sb[:].bitcast(I16), ps_s[h][:],
                                FE_C1, FE_C2,
                                mybir.AluOpType.mult, mybir.AluOpType.add)
                        else:
                            nc.scalar.activation(e_sb[:], ps_s[h][:], EXP,
                                                 scale=float(SCALE))
                        e_tiles[hp][sg] = e_sb
                side_flush(qt, p)
                prev_pair = (qt, p, e_tiles)
            # tail: drain the last pair and emit the final out-projection
            drain_pair(prev_pair, 0)
            for m in range(DT):
                outproj_group(NB - 1, m)
    nc.finalize()
    return nc


def kernel(query, key, value, mask, W_q, W_k, W_v, W_o):
    global _NC
    if _NC is None:
        _NC = _build()
    BF = ml_dtypes.bfloat16
    query = np.asarray(query, dtype=np.float32)
    key = np.asarray(key, dtype=np.float32)
    value = np.asarray(value, dtype=np.float32)
    W_q = np.asarray(W_q, dtype=np.float32)
    W_k = np.asarray(W_k, dtype=np.float32)
    W_v = np.asarray(W_v, dtype=np.float32)
    W_o = np.asarray(W_o, dtype=np.float32)
    mask = np.asarray(mask)
    ident = np.eye(128, dtype=np.float16)

    in_maps = []
    for c in range(NC_CORES):
        b, g = divmod(c, 2)
        hs = slice(g * CW, (g + 1) * CW)
        mrow = (mask[b, 0, 0, :] != 0).astype(np.float32)
        in_maps.append({
            "xqT": np.ascontiguousarray(query[b].T).astype(BF),
            "xkT": np.ascontiguousarray(key[b].T).astype(BF),
            "xvT": np.ascontiguousarray(value[b].T).astype(BF),
            "wqT": np.ascontiguousarray(W_q[hs, :].T).astype(BF),
            "wkT": np.ascontiguousarray(W_k[hs, :].T).astype(BF),
            "wvT": np.ascontiguousarray(W_v[hs, :].T).astype(BF),
            "woT": np.ascontiguousarray(W_o[:, hs].T).astype(np.float16),
            "maskf": np.ascontiguousarray(mrow.reshape(KT, 128).T),
            "ident": ident,
        })
    res = run_bass_kernel_spmd(_NC, in_maps, core_ids=list(range(NC_CORES)))
    out = np.empty((B, S, DM), np.float32)
    for b in range(B):
        out[b] = (res.results[2 * b]["outT"] + res.results[2 * b + 1]["outT"]).T
    return out


# revision 18
# speedup vs baseline: 1.0863x; 1.0672x over previous
"""MultiHeadAttention Trainium2 kernel.

Sharding: 8 cores = 4 batches x 2 head-groups (8 heads each).
Each core computes, for its (batch b, head-group g):
  Q^T = Wq_g @ Xq^T, K^T = Wk_g @ Xk^T   (bf16 matmuls, [headdim, S] fp16 tiles)
  V   = Xv @ Wv_g^T                       ([S, 512] fp16, +ones col, mask-scaled)
  scores^T[k,q] per head (K=64 fp16 matmuls), e = exp(s/8) -> fp16
  attnV in x-layout: ps_x[q=128, 65] += e_tile[k,q]^T @ [V|1][k,65]
    (full 128x128 PE utilization; one accumulation group per PSUM slot),
  normalize via per-partition reciprocal+tensor_scalar (denominator is the
  65th column), PE-transpose x back to [hd, q], out^T_partial = Wo_g^T.T @ x^T.
Host sums the two head-group partials per batch and transposes back.

attnV runs with a one-PAIR software lag: pair p's attnV groups are emitted
during pair p+1's supergroup slots (one complete 16-matmul accumulation
group per slot), with e tiles of pair p retained in SBUF.

Mask handling: V rows and the ones column are multiplied by mask (0/1), which
masks both the attnV numerator and the softmax denominator exactly.
"""
import contextlib
import os

import numpy as np
import ml_dtypes
import concourse.bass as bass  # noqa: F401
import concourse.tile as tile
from concourse import bacc, mybir
from concourse.bass_utils import run_bass_kernel_spmd

F32 = mybir.dt.float32
BF16 = mybir.dt.bfloat16
F16 = mybir.dt.float16
I16 = mybir.dt.int16
EXP = mybir.ActivationFunctionType.Exp

B, S, DM = 4, 2048, 1024
H = 16
DK = 64
HLOC = 8              # heads per core
CW = HLOC * DK        # 512 local head dims per core
NC_CORES = 8
KT = S // 128         # 16 k-tiles
NB = S // 512         # 4 q/s blocks of 512
MT = CW // 128        # 4 m-tiles of local head dims
DT = DM // 128        # 8 contraction tiles over d_model
SCALE = 1.0 / np.sqrt(DK)
SGW = 2               # k-tiles per supergroup
NSG = KT // SGW       # 8 supergroups

_NC = None


def _env(k, d):
    return int(os.environ.get(k, d))


def _build():
    nc = bacc.Bacc()
    xqT = nc.dram_tensor("xqT", [DM, S], BF16, kind="ExternalInput")
    xkT = nc.dram_tensor("xkT", [DM, S], BF16, kind="ExternalInput")
    xvT = nc.dram_tensor("xvT", [DM, S], BF16, kind="ExternalInput")
    wqT = nc.dram_tensor("wqT", [DM, CW], BF16, kind="ExternalInput")
    wkT = nc.dram_tensor("wkT", [DM, CW], BF16, kind="ExternalInput")
    wvT = nc.dram_tensor("wvT", [DM, CW], BF16, kind="ExternalInput")
    woT = nc.dram_tensor("woT", [CW, DM], F16, kind="ExternalInput")
    maskf = nc.dram_tensor("maskf", [128, KT], F32, kind="ExternalInput")
    ident = nc.dram_tensor("ident", [128, 128], F16, kind="ExternalInput")
    outT = nc.dram_tensor("outT", [DM, S], F32, kind="ExternalOutput")
    xdbg = (nc.dram_tensor("xdbg", [NB, 128, MT, 512], F16,
                           kind="ExternalOutput")
            if _env("K_DEBUG_X", 0) else None)

    # DVE fast-exp (int16 bit trick) constants: i16 = s*c1 + c2 bitcast f16
    FE_C1 = float(SCALE * np.log2(np.e) * 1024.0)
    FE_C2 = float(15.0 * 1024.0 - 486411.0 / 8192.0 + _env("K_FE_HALF", 0) * 0.5)
    EXPDVE_MOD = _env("K_EXPDVE_MOD", 6)   # offload exp of sg%MOD==MOD-1 to DVE

    with tile.TileContext(nc) as tc, contextlib.ExitStack() as ctx:
        persist = ctx.enter_context(tc.tile_pool(name="persist", bufs=1))

        # --- persistent tiles: mask, identity, Q^T/K^T slices, V ---
        m_sb = persist.tile([128, KT], F32)
        nc.sync.dma_start(m_sb[:], maskf[:])
        id16 = persist.tile([128, 128], F16, tag="id")
        nc.sync.dma_start(id16[:], ident[:])
        ones8 = persist.tile([128, HLOC], F32)
        nc.vector.memset(ones8[:], 1.0)
        warm = persist.tile([1, 1], F32)
        nc.scalar.activation(warm[:], ones8[0:1, 0:1], EXP, scale=1.0)
        q_tiles = {}   # (m, nb) -> [128, 512] f16  (Q^T slice)
        k_tiles = {}
        for m in range(MT):
            for n in range(NB):
                q_tiles[(m, n)] = persist.tile(
                    [128, 512], F16, tag=f"q{m}_{n}", name=f"q{m}_{n}")
                k_tiles[(m, n)] = persist.tile(
                    [128, 512], F16, tag=f"k{m}_{n}", name=f"k{m}_{n}")
        v_sb = persist.tile([128, KT, HLOC, DK + 1], F16, tag="v")

        # ---------------- Phase A: projections ----------------
        wq_pool = ctx.enter_context(tc.tile_pool(name="wqp", bufs=1))
        xt = ctx.enter_context(tc.tile_pool(name="xt", bufs=_env("K_XT_BUFS", 3)))
        ctxA = contextlib.ExitStack()
        with ctxA:
            wkv_pool = ctxA.enter_context(tc.tile_pool(name="wkv", bufs=1))
            psA = ctxA.enter_context(tc.tile_pool(name="psA", bufs=8, space="PSUM"))
            wq_t = wq_pool.tile([128, DT, CW], BF16, tag="wq")
            wk_t = wkv_pool.tile([128, DT, CW], BF16, tag="wk")
            wv_t = wq_pool.tile([128, DT, CW], BF16, tag="wv")
            wq_sb = [wq_t[:, k, :] for k in range(DT)]
            wk_sb = [wk_t[:, k, :] for k in range(DT)]
            wv_sb = [wv_t[:, k, :] for k in range(DT)]
            wo_all = persist.tile([128, MT, DM], F16, tag="wo")
            wo_t = [wo_all[:, k, :] for k in range(MT)]

            def dma_block(src, n, nm, wtiles=None, wsrc=None):
                blk = xt.tile([128, DT, 512], BF16, tag="xt",
                              name=f"{nm}{n}")
                nc.sync.dma_start(
                    blk[:],
                    src[:, n * 512:(n + 1) * 512].rearrange(
                        "(k p) c -> p k c", p=128))
                if wtiles is not None:
                    nc.sync.dma_start(
                        wtiles,
                        wsrc[:, :].rearrange("(k p) c -> p k c", p=128))
                return [blk[:, k, :] for k in range(DT)]

            def proj_group(dst_tiles, w_sb, xts, n, m, pool, tag):
                ps = pool.tile([128, 512], F32, tag=tag, name=f"pj{n}_{m}_{tag}")
                for k in range(DT):
                    nc.tensor.matmul(
                        ps[:], w_sb[k][:, m * 128:(m + 1) * 128],
                        xts[k][:], start=(k == 0), stop=(k == DT - 1))
                nc.vector.tensor_copy(dst_tiles[(m, n)][:], ps[:])

            def proj_block(dst_tiles, w_sb, wt, src, n, nm, wsrc=None):
                xts = dma_block(src, n, nm,
                                wtiles=wt if wsrc is not None else None,
                                wsrc=wsrc)
                for m in range(MT):
                    proj_group(dst_tiles, w_sb, xts, n, m, psA, "pa")

            def v_group(n, sm, xts, pool, tag):
                t = n * 4 + sm
                ps = pool.tile([128, 512], F32, tag=tag, name=f"vps{n}_{sm}")
                for k in range(DT):
                    nc.tensor.matmul(
                        ps[:], xts[k][:, sm * 128:(sm + 1) * 128],
                        wv_sb[k][:], start=(k == 0), stop=(k == DT - 1))
                # evacuate with mask scaling; set+mask ones column
                nc.vector.tensor_scalar_mul(
                    v_sb[:, t, :, 0:DK],
                    ps[:].rearrange("p (h d) -> p h d", h=HLOC),
                    m_sb[:, t:t + 1])
                nc.vector.tensor_scalar_mul(
                    v_sb[:, t, :, DK:DK + 1], ones8[:],
                    m_sb[:, t:t + 1])

            def v_block(n, pool, tag, first=False):
                xts = dma_block(xvT, n, "xv",
                                wtiles=wv_t[:] if first else None,
                                wsrc=wvT if first else None)
                for sm in range(4):        # s-tiles within block
                    v_group(n, sm, xts, pool, tag)

            # PE warmup: dummy matmuls cover initial DMA latency and start
            # the HAM activity window before the first real matmul.
            dum = wq_pool.tile([128, 512], BF16, tag="dum")
            nc.vector.memset(dum[:], 0.0)
            for i in range(_env("K_WARM_MM", 14)):
                pw = psA.tile([128, 512], F32, tag="pa", name=f"warmmm{i}")
                nc.tensor.matmul(pw[:], dum[:, 0:128], dum[:],
                                 start=True, stop=True)
            proj_block(k_tiles, wk_sb, wk_t[:], xkT, 0, "xk", wsrc=wkT)
            for n in range(1, NB):
                proj_block(k_tiles, wk_sb, wk_t[:], xkT, n, "xk")
            proj_block(q_tiles, wq_sb, wq_t[:], xqT, 0, "xq", wsrc=wqT)
            v_block(0, psA, "pa", first=True)
            v_block(1, psA, "pa")
            nc.sync.dma_start(
                wo_all[:], woT[:, :].rearrange("(k p) c -> p k c", p=128))

        # ---------------- Phase B: attention + out-proj ----------------
        with tc.tile_pool(name="ev", bufs=_env("K_EV_BUFS", 36)) as ev, \
             tc.tile_pool(name="x", bufs=2) as xpool, \
             tc.tile_pool(name="small", bufs=_env("K_SMALL_BUFS", 4)) as small, \
             tc.tile_pool(name="o", bufs=2) as opool, \
             tc.tile_pool(name="psS", bufs=_env("K_PSS_BUFS", 3), space="PSUM") as psS, \
             tc.tile_pool(name="psX", bufs=1, space="PSUM") as psX:
            # one persistent PSUM accumulator; slice g = group (qc, hp).
            # Padded to a 128-f32 group stride so no slice straddles a
            # 2KB PSUM bank boundary (matmul outs must stay in one bank).
            px_all = psX.tile([128, 2 * MT, DK + 1], F32, tag="xo",
                              padded_shape=[128, 2 * MT, 128])
            # x in q-partition layout, per qt: 4 tiles [128 q, 512 hd] f16
            x_sb = [[xpool.tile([128, 512], F16, tag=f"xs{qc}",
                                name=f"xs{i}_{qc}") for qc in range(4)]
                    for i in range(2)]
            # x^T tiles for outproj, per qt: [128 hd, MT, 512 q] f16
            xT_tiles = [xpool.tile([128, MT, 512], F16, tag="xT",
                                   name=f"xT{i}") for i in range(2)]

            def outproj_group(oqt, m):
                xT_prev = xT_tiles[oqt % 2]
                po = psS.tile([128, 512], F32, tag="s", name=f"po{oqt}_{m}")
                for kk in range(MT):
                    nc.tensor.matmul(
                        po[:], wo_t[kk][:, m * 128:(m + 1) * 128],
                        xT_prev[:, kk, :], start=(kk == 0), stop=(kk == MT - 1))
                o_sb = opool.tile([128, 512], F32, tag="ob")
                nc.vector.tensor_copy(o_sb[:], po[:])
                nc.sync.dma_start(
                    outT[m * 128:(m + 1) * 128, oqt * 512:(oqt + 1) * 512],
                    o_sb[:])

            # side-work: one psS-slot matmul group (or a DMA batch) per sg
            # step. (qt0,p0): v-blocks 2,3. (qt0,p1..3): late q projections.
            # (qt>0,p): out-projection of qt-1.
            xts_store = {}

            def mk_vdma(nn):
                def f():
                    xts_store[("v", nn)] = dma_block(xvT, nn, "xv")
                return ("dma", f)

            def mk_vg(nn, sm):
                return ("mm", lambda: v_group(nn, sm, xts_store[("v", nn)],
                                              psS, "s"))

            def mk_qdma(nn):
                def f():
                    xts_store[("q", nn)] = dma_block(xqT, nn, "xq")
                return ("dma", f)

            def mk_qg(nn, m):
                return ("mm", lambda: proj_group(q_tiles, wq_sb,
                                                 xts_store[("q", nn)],
                                                 nn, m, psS, "s"))

            side_work = {}
            side_work[(0, 0)] = [
                (0, mk_vdma(2)), (1, mk_vdma(3)),
                (1, mk_vg(2, 0)), (2, mk_vg(2, 1)),
                (3, mk_vg(2, 2)), (4, mk_vg(2, 3)),
                (5, mk_vg(3, 0)), (6, mk_vg(3, 1)),
                (7, mk_vg(3, 2)), (99, mk_vg(3, 3)),
            ]
            QOFF = _env("K_QOFF", 3)
            for n in range(1, NB):
                side_work[(0, n)] = [(0, mk_qdma(n))] + [
                    (QOFF + m, mk_qg(n, m)) for m in range(MT)]

            # out-projection of qt-1 spread over pairs p=1..3 of qt (the
            # last transposes of (qt-1, 3) are only emitted during (qt, 0)).
            OPROJ_M = {1: (0, 1, 2), 2: (3, 4, 5), 3: (6, 7)}

            def side_step(qt, p, sg):
                if qt > 0 and p >= 1:
                    ms = OPROJ_M[p]
                    off = _env("K_OOFF", 1)
                    if sg >= off and (sg - off) % 2 == 0:
                        idx = (sg - off) // 2
                        if idx < len(ms):
                            outproj_group(qt - 1, ms[idx])
                    return
                work = side_work.get((qt, p))
                if not work:
                    return
                did_mm = False
                while work:
                    min_sg, (kind, fn) = work[0]
                    if min_sg > sg or (kind == "mm" and did_mm):
                        break
                    work.pop(0)
                    fn()
                    if kind == "mm":
                        did_mm = True

            def side_flush(qt, p):
                for _, (kind, fn) in side_work.pop((qt, p), []):
                    fn()

            # Pair pipeline: each slot of pair (qt,p) also runs one complete
            # attnV accumulation group of the PREVIOUS pair, plus its norm.
            # prev_pair state: (qt, p, e_tiles{h: [tile per sg]}).
            prev_pair = None

            def attnv_group(pqt, pp, e_tiles, g):
                """Emit accumulation group g (=(qc, hp)) of pair pp: 16
                matmuls into one psX slot, then normalize into x_sb."""
                qc, hp = divmod(g, 2)
                h = 2 * pp + hp
                px = px_all[:, g, :]
                for t in range(KT):
                    nc.tensor.matmul(
                        px,
                        e_tiles[hp][t // SGW][:, t % SGW,
                                              qc * 128:(qc + 1) * 128],
                        v_sb[:, t, h, :],
                        start=(t == 0), stop=(t == KT - 1))
                r = small.tile([128, 1], F32, tag="r",
                               name=f"r{pqt}_{pp}_{g}")
                nc.vector.reciprocal(r[:], px[:, DK:DK + 1])
                nc.vector.tensor_scalar(
                    x_sb[pqt % 2][qc][:, pp * 128 + hp * 64:
                                      pp * 128 + (hp + 1) * 64],
                    px[:, 0:DK], r[:], None, mybir.AluOpType.mult)

            def transpose_block(pqt, pp, qc):
                """x_sb[qc][:, pp*128:+128] -> xT[:, pp, qc*128:+128]."""
                pt = psS.tile([128, 128], F16, tag="s",
                              name=f"tr{pqt}_{pp}_{qc}")
                nc.tensor.transpose(
                    pt[:], x_sb[pqt % 2][qc][:, pp * 128:(pp + 1) * 128],
                    id16[:])
                nc.vector.tensor_copy(
                    xT_tiles[pqt % 2][:, pp, qc * 128:(qc + 1) * 128], pt[:])
                if xdbg is not None:
                    nc.sync.dma_start(
                        xdbg[pqt, :, pp, qc * 128:(qc + 1) * 128],
                        xT_tiles[pqt % 2][:, pp, qc * 128:(qc + 1) * 128])

            def drain_pair(pair, sg_from):
                """Emit remaining attnV groups + transposes of `pair`."""
                if pair is None:
                    return
                pqt, pp, e_tiles = pair
                for g in range(sg_from, NSG):
                    attnv_group(pqt, pp, e_tiles, g)
                for qc in range(4):
                    transpose_block(pqt, pp, qc)

            pair_list = [(qt, p) for qt in range(NB) for p in range(MT)]
            for qt, p in pair_list:
                heads = (2 * p, 2 * p + 1)
                e_tiles = {0: [None] * NSG, 1: [None] * NSG}
                for sg in range(NSG):
                    ps_s = {h: psS.tile([128, SGW, 512], F32, tag="s",
                                        name=f"pss{qt}_{sg}_{h}")
                            for h in heads}
                    side_step(qt, p, sg)
                    for tt in range(SGW):
                        t = sg * SGW + tt
                        for h in heads:
                            hp = h % 2
                            nc.tensor.matmul(
                                ps_s[h][:, tt, :],
                                k_tiles[(p, t // 4)][
                                    hp * 64:(hp + 1) * 64,
                                    (t % 4) * 128:(t % 4 + 1) * 128],
                                q_tiles[(p, qt)][hp * 64:(hp + 1) * 64, :],
                                start=True, stop=True)
                    # one attnV group of the previous pair per slot
                    if prev_pair is not None:
                        attnv_group(prev_pair[0], prev_pair[1],
                                    prev_pair[2], sg)
                        # transposes of the previous pair trail the groups
                        if sg >= 2 and sg % 2 == 0:
                            transpose_block(prev_pair[0], prev_pair[1],
                                            sg // 2 - 1)
                        elif sg == NSG - 1:
                            transpose_block(prev_pair[0], prev_pair[1], 3)
                    for h in heads:
                        hp = h % 2
                        e_sb = ev.tile([128, SGW, 512], F16, tag="e",
                                       name=f"e{qt}_{p}_{sg}_{h}")
                        if EXPDVE_MOD and sg % EXPDVE_MOD == EXPDVE_MOD - 1:
                            nc.vector.tensor_scalar(
                                e_sb[:].bitcast(I16), ps_s[h][:],
                                FE_C1, FE_C2,
                                mybir.AluOpType.mult, mybir.AluOpType.add)
                        else:
                            nc.scalar.activation(e_sb[:], ps_s[h][:], EXP,
                                                 scale=float(SCALE))
                        e_tiles[hp][sg] = e_sb
                side_flush(qt, p)
                prev_pair = (qt, p, e_tiles)
            # tail: drain the last pair and emit the final out-projection
            drain_pair(prev_pair, 0)
            for m in range(DT):
                outproj_group(NB - 1, m)
    nc.finalize()
    return nc


def kernel(query, key, value, mask, W_q, W_k, W_v, W_o):
    global _NC
    if _NC is None:
        _NC = _build()
    BF = ml_dtypes.bfloat16
    query = np.asarray(query, dtype=np.float32)
    key = np.asarray(key, dtype=np.float32)
    value = np.asarray(value, dtype=np.float32)
    W_q = np.asarray(W_q, dtype=np.float32)
    W_k = np.asarray(W_k, dtype=np.float32)
    W_v = np.asarray(W_v, dtype=np.float32)
    W_o = np.asarray(W_o, dtype=np.float32)
    mask = np.asarray(mask)
    ident = np.eye(128, dtype=np.float16)

    in_maps = []
    for c in range(NC_CORES):
        b, g = divmod(c, 2)
        hs = slice(g * CW, (g + 1) * CW)
        mrow = (mask[b, 0, 0, :] != 0).astype(np.float32)
        in_maps.append({
            "xqT": np.ascontiguousarray(query[b].T).astype(BF),
            "xkT": np.ascontiguousarray(key[b].T).astype(BF),
            "xvT": np.ascontiguousarray(value[b].T).astype(BF),
            "wqT": np.ascontiguousarray(W_q[hs, :].T).astype(BF),
            "wkT": np.ascontiguousarray(W_k[hs, :].T).astype(BF),
            "wvT": np.ascontiguousarray(W_v[hs, :].T).astype(BF),
            "woT": np.ascontiguousarray(W_o[:, hs].T).astype(np.float16),
            "maskf": np.ascontiguousarray(mrow.reshape(KT, 128).T),
            "ident": ident,
        })
    res = run_bass_kernel_spmd(_NC, in_maps, core_ids=list(range(NC_CORES)))
    out = np.empty((B, S, DM), np.float32)
    for b in range(B):
        out[b] = (res.results[2 * b]["outT"] + res.results[2 * b + 1]["outT"]).T
    return out


# revision 20
# speedup vs baseline: 1.0906x; 1.0039x over previous
"""MultiHeadAttention Trainium2 kernel.

Sharding: 8 cores = 4 batches x 2 head-groups (8 heads each).
Each core computes, for its (batch b, head-group g):
  Q^T = Wq_g @ Xq^T, K^T = Wk_g @ Xk^T   (bf16 matmuls, [headdim, S] fp16 tiles)
  V   = Xv @ Wv_g^T                       ([S, 512] fp16, +ones col, mask-scaled)
  scores^T[k,q] per head (K=64 fp16 matmuls), e = exp(s/8) -> fp16
  attnV in x-layout: ps_x[q=128, 65] += e_tile[k,q]^T @ [V|1][k,65]
    (full 128x128 PE utilization; one accumulation group per PSUM slot),
  normalize via per-partition reciprocal+tensor_scalar (denominator is the
  65th column), PE-transpose x back to [hd, q], out^T_partial = Wo_g^T.T @ x^T.
Host sums the two head-group partials per batch and transposes back.

attnV runs with a one-PAIR software lag: pair p's attnV groups are emitted
during pair p+1's supergroup slots (one complete 16-matmul accumulation
group per slot), with e tiles of pair p retained in SBUF.

Mask handling: V rows and the ones column are multiplied by mask (0/1), which
masks both the attnV numerator and the softmax denominator exactly.
"""
import contextlib
import os

import numpy as np
import ml_dtypes
import concourse.bass as bass  # noqa: F401
import concourse.tile as tile
from concourse import bacc, mybir
from concourse.bass_utils import run_bass_kernel_spmd

F32 = mybir.dt.float32
BF16 = mybir.dt.bfloat16
F16 = mybir.dt.float16
I16 = mybir.dt.int16
EXP = mybir.ActivationFunctionType.Exp

B, S, DM = 4, 2048, 1024
H = 16
DK = 64
HLOC = 8              # heads per core
CW = HLOC * DK        # 512 local head dims per core
NC_CORES = 8
KT = S // 128         # 16 k-tiles
NB = S // 512         # 4 q/s blocks of 512
MT = CW // 128        # 4 m-tiles of local head dims
DT = DM // 128        # 8 contraction tiles over d_model
SCALE = 1.0 / np.sqrt(DK)
SGW = 2               # k-tiles per supergroup
NSG = KT // SGW       # 8 supergroups

_NC = None


def _env(k, d):
    return int(os.environ.get(k, d))


def _build():
    nc = bacc.Bacc()
    xqT = nc.dram_tensor("xqT", [DM, S], BF16, kind="ExternalInput")
    xkT = nc.dram_tensor("xkT", [DM, S], BF16, kind="ExternalInput")
    xvT = nc.dram_tensor("xvT", [DM, S], BF16, kind="ExternalInput")
    wqT = nc.dram_tensor("wqT", [DM, CW], BF16, kind="ExternalInput")
    wkT = nc.dram_tensor("wkT", [DM, CW], BF16, kind="ExternalInput")
    wvT = nc.dram_tensor("wvT", [DM, CW], BF16, kind="ExternalInput")
    woT = nc.dram_tensor("woT", [CW, DM], F16, kind="ExternalInput")
    maskf = nc.dram_tensor("maskf", [128, KT], F32, kind="ExternalInput")
    ident = nc.dram_tensor("ident", [128, 128], F16, kind="ExternalInput")
    outT = nc.dram_tensor("outT", [DM, S], F32, kind="ExternalOutput")
    xdbg = (nc.dram_tensor("xdbg", [NB, 128, MT, 512], F16,
                           kind="ExternalOutput")
            if _env("K_DEBUG_X", 0) else None)

    # DVE fast-exp (int16 bit trick) constants: i16 = s*c1 + c2 bitcast f16
    FE_C1 = float(SCALE * np.log2(np.e) * 1024.0)
    FE_C2 = float(15.0 * 1024.0 - 486411.0 / 8192.0 + _env("K_FE_HALF", 0) * 0.5)
    EXPDVE_MOD = _env("K_EXPDVE_MOD", 3)   # offload exp of sg%MOD==MOD-1 to DVE

    with tile.TileContext(nc) as tc, contextlib.ExitStack() as ctx:
        persist = ctx.enter_context(tc.tile_pool(name="persist", bufs=1))

        # --- persistent tiles: mask, identity, Q^T/K^T slices, V ---
        m_sb = persist.tile([128, KT], F32)
        nc.sync.dma_start(m_sb[:], maskf[:])
        id16 = persist.tile([128, 128], F16, tag="id")
        nc.sync.dma_start(id16[:], ident[:])
        ones8 = persist.tile([128, HLOC], F32)
        nc.vector.memset(ones8[:], 1.0)
        warm = persist.tile([1, 1], F32)
        nc.scalar.activation(warm[:], ones8[0:1, 0:1], EXP, scale=1.0)
        q_tiles = {}   # (m, nb) -> [128, 512] f16  (Q^T slice)
        k_tiles = {}
        for m in range(MT):
            for n in range(NB):
                q_tiles[(m, n)] = persist.tile(
                    [128, 512], F16, tag=f"q{m}_{n}", name=f"q{m}_{n}")
                k_tiles[(m, n)] = persist.tile(
                    [128, 512], F16, tag=f"k{m}_{n}", name=f"k{m}_{n}")
        v_sb = persist.tile([128, KT, HLOC, DK + 1], F16, tag="v")

        # ---------------- Phase A: projections ----------------
        wq_pool = ctx.enter_context(tc.tile_pool(name="wqp", bufs=1))
        xt = ctx.enter_context(tc.tile_pool(name="xt", bufs=_env("K_XT_BUFS", 3)))
        ctxA = contextlib.ExitStack()
        with ctxA:
            wkv_pool = ctxA.enter_context(tc.tile_pool(name="wkv", bufs=1))
            psA = ctxA.enter_context(tc.tile_pool(name="psA", bufs=8, space="PSUM"))
            wq_t = wq_pool.tile([128, DT, CW], BF16, tag="wq")
            wk_t = wkv_pool.tile([128, DT, CW], BF16, tag="wk")
            wv_t = wq_pool.tile([128, DT, CW], BF16, tag="wv")
            wq_sb = [wq_t[:, k, :] for k in range(DT)]
            wk_sb = [wk_t[:, k, :] for k in range(DT)]
            wv_sb = [wv_t[:, k, :] for k in range(DT)]
            wo_all = persist.tile([128, MT, DM], F16, tag="wo")
            wo_t = [wo_all[:, k, :] for k in range(MT)]

            def dma_block(src, n, nm, wtiles=None, wsrc=None):
                blk = xt.tile([128, DT, 512], BF16, tag="xt",
                              name=f"{nm}{n}")
                nc.sync.dma_start(
                    blk[:],
                    src[:, n * 512:(n + 1) * 512].rearrange(
                        "(k p) c -> p k c", p=128))
                if wtiles is not None:
                    nc.sync.dma_start(
                        wtiles,
                        wsrc[:, :].rearrange("(k p) c -> p k c", p=128))
                return [blk[:, k, :] for k in range(DT)]

            def proj_group(dst_tiles, w_sb, xts, n, m, pool, tag):
                ps = pool.tile([128, 512], F32, tag=tag, name=f"pj{n}_{m}_{tag}")
                for k in range(DT):
                    nc.tensor.matmul(
                        ps[:], w_sb[k][:, m * 128:(m + 1) * 128],
                        xts[k][:], start=(k == 0), stop=(k == DT - 1))
                nc.vector.tensor_copy(dst_tiles[(m, n)][:], ps[:])

            def proj_block(dst_tiles, w_sb, wt, src, n, nm, wsrc=None):
                xts = dma_block(src, n, nm,
                                wtiles=wt if wsrc is not None else None,
                                wsrc=wsrc)
                for m in range(MT):
                    proj_group(dst_tiles, w_sb, xts, n, m, psA, "pa")

            def v_group(n, sm, xts, pool, tag):
                t = n * 4 + sm
                ps = pool.tile([128, 512], F32, tag=tag, name=f"vps{n}_{sm}")
                for k in range(DT):
                    nc.tensor.matmul(
                        ps[:], xts[k][:, sm * 128:(sm + 1) * 128],
                        wv_sb[k][:], start=(k == 0), stop=(k == DT - 1))
                # evacuate with mask scaling; set+mask ones column
                nc.vector.tensor_scalar_mul(
                    v_sb[:, t, :, 0:DK],
                    ps[:].rearrange("p (h d) -> p h d", h=HLOC),
                    m_sb[:, t:t + 1])
                nc.vector.tensor_scalar_mul(
                    v_sb[:, t, :, DK:DK + 1], ones8[:],
                    m_sb[:, t:t + 1])

            def v_block(n, pool, tag, first=False):
                xts = dma_block(xvT, n, "xv",
                                wtiles=wv_t[:] if first else None,
                                wsrc=wvT if first else None)
                for sm in range(4):        # s-tiles within block
                    v_group(n, sm, xts, pool, tag)

            # PE warmup: dummy matmuls cover initial DMA latency and start
            # the HAM activity window before the first real matmul.
            dum = wq_pool.tile([128, 512], BF16, tag="dum")
            nc.vector.memset(dum[:], 0.0)
            for i in range(_env("K_WARM_MM", 14)):
                pw = psA.tile([128, 512], F32, tag="pa", name=f"warmmm{i}")
                nc.tensor.matmul(pw[:], dum[:, 0:128], dum[:],
                                 start=True, stop=True)
            proj_block(k_tiles, wk_sb, wk_t[:], xkT, 0, "xk", wsrc=wkT)
            for n in range(1, NB):
                proj_block(k_tiles, wk_sb, wk_t[:], xkT, n, "xk")
            proj_block(q_tiles, wq_sb, wq_t[:], xqT, 0, "xq", wsrc=wqT)
            v_block(0, psA, "pa", first=True)
            v_block(1, psA, "pa")
            nc.sync.dma_start(
                wo_all[:], woT[:, :].rearrange("(k p) c -> p k c", p=128))

        # ---------------- Phase B: attention + out-proj ----------------
        with tc.tile_pool(name="ev", bufs=_env("K_EV_BUFS", 36)) as ev, \
             tc.tile_pool(name="x", bufs=2) as xpool, \
             tc.tile_pool(name="small", bufs=_env("K_SMALL_BUFS", 4)) as small, \
             tc.tile_pool(name="o", bufs=2) as opool, \
             tc.tile_pool(name="psS", bufs=_env("K_PSS_BUFS", 3), space="PSUM") as psS, \
             tc.tile_pool(name="psX", bufs=1, space="PSUM") as psX:
            # one persistent PSUM accumulator; slice g = group (qc, hp).
            # Padded to a 128-f32 group stride so no slice straddles a
            # 2KB PSUM bank boundary (matmul outs must stay in one bank).
            px_all = psX.tile([128, 2 * MT, DK + 1], F32, tag="xo",
                              padded_shape=[128, 2 * MT, 128])
            # x in q-partition layout, per qt: 4 tiles [128 q, 512 hd] f16
            x_sb = [[xpool.tile([128, 512], F16, tag=f"xs{qc}",
                                name=f"xs{i}_{qc}") for qc in range(4)]
                    for i in range(2)]
            # x^T tiles for outproj, per qt: [128 hd, MT, 512 q] f16
            xT_tiles = [xpool.tile([128, MT, 512], F16, tag="xT",
                                   name=f"xT{i}") for i in range(2)]

            def outproj_group(oqt, m):
                xT_prev = xT_tiles[oqt % 2]
                po = psS.tile([128, 512], F32, tag="s", name=f"po{oqt}_{m}")
                for kk in range(MT):
                    nc.tensor.matmul(
                        po[:], wo_t[kk][:, m * 128:(m + 1) * 128],
                        xT_prev[:, kk, :], start=(kk == 0), stop=(kk == MT - 1))
                o_sb = opool.tile([128, 512], F32, tag="ob")
                nc.vector.tensor_copy(o_sb[:], po[:])
                nc.sync.dma_start(
                    outT[m * 128:(m + 1) * 128, oqt * 512:(oqt + 1) * 512],
                    o_sb[:])

            # side-work: one psS-slot matmul group (or a DMA batch) per sg
            # step. (qt0,p0): v-blocks 2,3. (qt0,p1..3): late q projections.
            # (qt>0,p): out-projection of qt-1.
            xts_store = {}

            def mk_vdma(nn):
                def f():
                    xts_store[("v", nn)] = dma_block(xvT, nn, "xv")
                return ("dma", f)

            def mk_vg(nn, sm):
                return ("mm", lambda: v_group(nn, sm, xts_store[("v", nn)],
                                              psS, "s"))

            def mk_qdma(nn):
                def f():
                    xts_store[("q", nn)] = dma_block(xqT, nn, "xq")
                return ("dma", f)

            def mk_qg(nn, m):
                return ("mm", lambda: proj_group(q_tiles, wq_sb,
                                                 xts_store[("q", nn)],
                                                 nn, m, psS, "s"))

            side_work = {}
            side_work[(0, 0)] = [
                (0, mk_vdma(2)), (1, mk_vdma(3)),
                (1, mk_vg(2, 0)), (2, mk_vg(2, 1)),
                (3, mk_vg(2, 2)), (4, mk_vg(2, 3)),
                (5, mk_vg(3, 0)), (6, mk_vg(3, 1)),
                (7, mk_vg(3, 2)), (99, mk_vg(3, 3)),
            ]
            QOFF = _env("K_QOFF", 3)
            for n in range(1, NB):
                side_work[(0, n)] = [(0, mk_qdma(n))] + [
                    (QOFF + m, mk_qg(n, m)) for m in range(MT)]

            # out-projection of qt-1 spread over pairs p=1..3 of qt (the
            # last transposes of (qt-1, 3) are only emitted during (qt, 0)).
            OPROJ_M = {1: (0, 1, 2), 2: (3, 4, 5), 3: (6, 7)}

            def side_step(qt, p, sg):
                if qt > 0 and p >= 1:
                    ms = OPROJ_M[p]
                    off = _env("K_OOFF", 1)
                    if sg >= off and (sg - off) % 2 == 0:
                        idx = (sg - off) // 2
                        if idx < len(ms):
                            outproj_group(qt - 1, ms[idx])
                    return
                work = side_work.get((qt, p))
                if not work:
                    return
                did_mm = False
                while work:
                    min_sg, (kind, fn) = work[0]
                    if min_sg > sg or (kind == "mm" and did_mm):
                        break
                    work.pop(0)
                    fn()
                    if kind == "mm":
                        did_mm = True

            def side_flush(qt, p):
                for _, (kind, fn) in side_work.pop((qt, p), []):
                    fn()

            # Pair pipeline: each slot of pair (qt,p) also runs one complete
            # attnV accumulation group of the PREVIOUS pair, plus its norm.
            # prev_pair state: (qt, p, e_tiles{h: [tile per sg]}).
            prev_pair = None

            def attnv_group(pqt, pp, e_tiles, g):
                """Emit accumulation group g (=(qc, hp)) of pair pp: 16
                matmuls into one psX slot, then normalize into x_sb."""
                qc, hp = divmod(g, 2)
                h = 2 * pp + hp
                px = px_all[:, g, :]
                for t in range(KT):
                    nc.tensor.matmul(
                        px,
                        e_tiles[hp][t // SGW][:, t % SGW,
                                              qc * 128:(qc + 1) * 128],
                        v_sb[:, t, h, :],
                        start=(t == 0), stop=(t == KT - 1))
                r = small.tile([128, 1], F32, tag="r",
                               name=f"r{pqt}_{pp}_{g}")
                nc.vector.reciprocal(r[:], px[:, DK:DK + 1])
                nc.vector.tensor_scalar(
                    x_sb[pqt % 2][qc][:, pp * 128 + hp * 64:
                                      pp * 128 + (hp + 1) * 64],
                    px[:, 0:DK], r[:], None, mybir.AluOpType.mult)

            def transpose_block(pqt, pp, qc):
                """x_sb[qc][:, pp*128:+128] -> xT[:, pp, qc*128:+128]."""
                pt = psS.tile([128, 128], F16, tag="s",
                              name=f"tr{pqt}_{pp}_{qc}")
                nc.tensor.transpose(
                    pt[:], x_sb[pqt % 2][qc][:, pp * 128:(pp + 1) * 128],
                    id16[:])
                nc.vector.tensor_copy(
                    xT_tiles[pqt % 2][:, pp, qc * 128:(qc + 1) * 128], pt[:])
                if xdbg is not None:
                    nc.sync.dma_start(
                        xdbg[pqt, :, pp, qc * 128:(qc + 1) * 128],
                        xT_tiles[pqt % 2][:, pp, qc * 128:(qc + 1) * 128])

            def drain_pair(pair, sg_from):
                """Emit remaining attnV groups + transposes of `pair`."""
                if pair is None:
                    return
                pqt, pp, e_tiles = pair
                for g in range(sg_from, NSG):
                    attnv_group(pqt, pp, e_tiles, g)
                for qc in range(4):
                    transpose_block(pqt, pp, qc)

            pair_list = [(qt, p) for qt in range(NB) for p in range(MT)]
            for qt, p in pair_list:
                heads = (2 * p, 2 * p + 1)
                e_tiles = {0: [None] * NSG, 1: [None] * NSG}
                for sg in range(NSG):
                    ps_s = {h: psS.tile([128, SGW, 512], F32, tag="s",
                                        name=f"pss{qt}_{sg}_{h}")
                            for h in heads}
                    side_step(qt, p, sg)
                    for tt in range(SGW):
                        t = sg * SGW + tt
                        for h in heads:
                            hp = h % 2
                            nc.tensor.matmul(
                                ps_s[h][:, tt, :],
                                k_tiles[(p, t // 4)][
                                    hp * 64:(hp + 1) * 64,
                                    (t % 4) * 128:(t % 4 + 1) * 128],
                                q_tiles[(p, qt)][hp * 64:(hp + 1) * 64, :],
                                start=True, stop=True)
                    # one attnV group of the previous pair per slot
                    if prev_pair is not None:
                        attnv_group(prev_pair[0], prev_pair[1],
                                    prev_pair[2], sg)
                        # transposes of the previous pair trail the groups
                        if sg >= 2 and sg % 2 == 0:
                            transpose_block(prev_pair[0], prev_pair[1],
                                            sg // 2 - 1)
                        elif sg == NSG - 1:
                            transpose_block(prev_pair[0], prev_pair[1], 3)
                    # exp: ACT mostly; offload a half-tile (one head, tt=1)
                    # to DVE fast-exp on 2-of-3 slots to balance per-slot
                    # ACT time against PE (total offload mass = 1/6).
                    off_h = None
                    if EXPDVE_MOD == 3 and sg % 3:
                        off_h = heads[sg % 3 - 1]
                    elif EXPDVE_MOD > 3 and sg % EXPDVE_MOD == EXPDVE_MOD - 1:
                        off_h = "all"
                    for h in heads:
                        hp = h % 2
                        e_sb = ev.tile([128, SGW, 512], F16, tag="e",
                                       name=f"e{qt}_{p}_{sg}_{h}")
                        if off_h == "all":
                            nc.vector.tensor_scalar(
                                e_sb[:].bitcast(I16), ps_s[h][:],
                                FE_C1, FE_C2,
                                mybir.AluOpType.mult, mybir.AluOpType.add)
                        elif off_h == h:
                            nc.scalar.activation(e_sb[:, 0, :],
                                                 ps_s[h][:, 0, :], EXP,
                                                 scale=float(SCALE))
                            nc.vector.tensor_scalar(
                                e_sb[:, 1, :].bitcast(I16), ps_s[h][:, 1, :],
                                FE_C1, FE_C2,
                                mybir.AluOpType.mult, mybir.AluOpType.add)
                        else:
                            nc.scalar.activation(e_sb[:], ps_s[h][:], EXP,
                                                 scale=float(SCALE))
                        e_tiles[hp][sg] = e_sb
                side_flush(qt, p)
                prev_pair = (qt, p, e_tiles)
            # tail: drain the last pair and emit the final out-projection
            drain_pair(prev_pair, 0)
            for m in range(DT):
                outproj_group(NB - 1, m)
    nc.finalize()
    return nc


def kernel(query, key, value, mask, W_q, W_k, W_v, W_o):
    global _NC
    if _NC is None:
        _NC = _build()
    BF = ml_dtypes.bfloat16
    query = np.asarray(query, dtype=np.float32)
    key = np.asarray(key, dtype=np.float32)
    value = np.asarray(value, dtype=np.float32)
    W_q = np.asarray(W_q, dtype=np.float32)
    W_k = np.asarray(W_k, dtype=np.float32)
    W_v = np.asarray(W_v, dtype=np.float32)
    W_o = np.asarray(W_o, dtype=np.float32)
    mask = np.asarray(mask)
    ident = np.eye(128, dtype=np.float16)

    in_maps = []
    for c in range(NC_CORES):
        b, g = divmod(c, 2)
        hs = slice(g * CW, (g + 1) * CW)
        mrow = (mask[b, 0, 0, :] != 0).astype(np.float32)
        in_maps.append({
            "xqT": np.ascontiguousarray(query[b].T).astype(BF),
            "xkT": np.ascontiguousarray(key[b].T).astype(BF),
            "xvT": np.ascontiguousarray(value[b].T).astype(BF),
            "wqT": np.ascontiguousarray(W_q[hs, :].T).astype(BF),
            "wkT": np.ascontiguousarray(W_k[hs, :].T).astype(BF),
            "wvT": np.ascontiguousarray(W_v[hs, :].T).astype(BF),
            "woT": np.ascontiguousarray(W_o[:, hs].T).astype(np.float16),
            "maskf": np.ascontiguousarray(mrow.reshape(KT, 128).T),
            "ident": ident,
        })
    res = run_bass_kernel_spmd(_NC, in_maps, core_ids=list(range(NC_CORES)))
    out = np.empty((B, S, DM), np.float32)
    for b in range(B):
        out[b] = (res.results[2 * b]["outT"] + res.results[2 * b + 1]["outT"]).T
    return out


# revision 21
# speedup vs baseline: 1.1041x; 1.0124x over previous
"""MultiHeadAttention Trainium2 kernel.

Sharding: 8 cores = 4 batches x 2 head-groups (8 heads each).
Each core computes, for its (batch b, head-group g):
  Q^T = Wq_g @ Xq^T, K^T = Wk_g @ Xk^T   (bf16 matmuls, [headdim, S] fp16 tiles)
  V   = Xv @ Wv_g^T                       ([S, 512] fp16, +ones col, mask-scaled)
  scores^T[k,q] per head (K=64 fp16 matmuls), e = exp(s/8) -> fp16
  attnV in x-layout: ps_x[q=128, 65] += e_tile[k,q]^T @ [V|1][k,65]
    (full 128x128 PE utilization; one accumulation group per PSUM slot),
  normalize via per-partition reciprocal+tensor_scalar (denominator is the
  65th column), PE-transpose x back to [hd, q], out^T_partial = Wo_g^T.T @ x^T.
Host sums the two head-group partials per batch and transposes back.

attnV runs with a one-PAIR software lag: pair p's attnV groups are emitted
during pair p+1's supergroup slots (one complete 16-matmul accumulation
group per slot), with e tiles of pair p retained in SBUF.

Mask handling: V rows and the ones column are multiplied by mask (0/1), which
masks both the attnV numerator and the softmax denominator exactly.
"""
import contextlib
import os

import numpy as np
import ml_dtypes
import concourse.bass as bass  # noqa: F401
import concourse.tile as tile
from concourse import bacc, mybir
from concourse.bass_utils import run_bass_kernel_spmd

F32 = mybir.dt.float32
BF16 = mybir.dt.bfloat16
F16 = mybir.dt.float16
I16 = mybir.dt.int16
EXP = mybir.ActivationFunctionType.Exp

B, S, DM = 4, 2048, 1024
H = 16
DK = 64
HLOC = 8              # heads per core
CW = HLOC * DK        # 512 local head dims per core
NC_CORES = 8
KT = S // 128         # 16 k-tiles
NB = S // 512         # 4 q/s blocks of 512
MT = CW // 128        # 4 m-tiles of local head dims
DT = DM // 128        # 8 contraction tiles over d_model
SCALE = 1.0 / np.sqrt(DK)
SGW = 2               # k-tiles per supergroup
NSG = KT // SGW       # 8 supergroups

_NC = None


def _env(k, d):
    return int(os.environ.get(k, d))


def _build():
    nc = bacc.Bacc()
    xqT = nc.dram_tensor("xqT", [DM, S], BF16, kind="ExternalInput")
    xkT = nc.dram_tensor("xkT", [DM, S], BF16, kind="ExternalInput")
    xvT = nc.dram_tensor("xvT", [DM, S], BF16, kind="ExternalInput")
    wqT = nc.dram_tensor("wqT", [DM, CW], BF16, kind="ExternalInput")
    wkT = nc.dram_tensor("wkT", [DM, CW], BF16, kind="ExternalInput")
    wvT = nc.dram_tensor("wvT", [DM, CW], BF16, kind="ExternalInput")
    woT = nc.dram_tensor("woT", [CW, DM], F16, kind="ExternalInput")
    maskf = nc.dram_tensor("maskf", [128, KT], F32, kind="ExternalInput")
    ident = nc.dram_tensor("ident", [128, 128], F16, kind="ExternalInput")
    outT = nc.dram_tensor("outT", [DM, S], F32, kind="ExternalOutput")
    xdbg = (nc.dram_tensor("xdbg", [NB, 128, MT, 512], F16,
                           kind="ExternalOutput")
            if _env("K_DEBUG_X", 0) else None)

    # DVE fast-exp (int16 bit trick) constants: i16 = s*c1 + c2 bitcast f16
    FE_C1 = float(SCALE * np.log2(np.e) * 1024.0)
    FE_C2 = float(15.0 * 1024.0 - 486411.0 / 8192.0 + _env("K_FE_HALF", 0) * 0.5)
    EXPDVE_MOD = _env("K_EXPDVE_MOD", 3)   # offload exp of sg%MOD==MOD-1 to DVE

    with tile.TileContext(nc) as tc, contextlib.ExitStack() as ctx:
        persist = ctx.enter_context(tc.tile_pool(name="persist", bufs=1))

        # --- persistent tiles: mask, identity, Q^T/K^T slices, V ---
        m_sb = persist.tile([128, KT], F32)
        nc.sync.dma_start(m_sb[:], maskf[:])
        id16 = persist.tile([128, 128], F16, tag="id")
        nc.sync.dma_start(id16[:], ident[:])
        ones8 = persist.tile([128, HLOC], F32)
        nc.vector.memset(ones8[:], 1.0)
        warm = persist.tile([1, 1], F32)
        nc.scalar.activation(warm[:], ones8[0:1, 0:1], EXP, scale=1.0)
        q_tiles = {}   # (m, nb) -> [128, 512] f16  (Q^T slice)
        k_tiles = {}
        for m in range(MT):
            for n in range(NB):
                q_tiles[(m, n)] = persist.tile(
                    [128, 512], F16, tag=f"q{m}_{n}", name=f"q{m}_{n}")
                k_tiles[(m, n)] = persist.tile(
                    [128, 512], F16, tag=f"k{m}_{n}", name=f"k{m}_{n}")
        v_sb = persist.tile([128, KT, HLOC, DK + 1], F16, tag="v")

        # ---------------- Phase A: projections ----------------
        wq_pool = ctx.enter_context(tc.tile_pool(name="wqp", bufs=1))
        xt = ctx.enter_context(tc.tile_pool(name="xt", bufs=_env("K_XT_BUFS", 3)))
        ctxA = contextlib.ExitStack()
        with ctxA:
            wkv_pool = ctxA.enter_context(tc.tile_pool(name="wkv", bufs=1))
            psA = ctxA.enter_context(tc.tile_pool(name="psA", bufs=8, space="PSUM"))
            wq_t = wq_pool.tile([128, DT, CW], BF16, tag="wq")
            wk_t = wkv_pool.tile([128, DT, CW], BF16, tag="wk")
            wv_t = wq_pool.tile([128, DT, CW], BF16, tag="wv")
            wq_sb = [wq_t[:, k, :] for k in range(DT)]
            wk_sb = [wk_t[:, k, :] for k in range(DT)]
            wv_sb = [wv_t[:, k, :] for k in range(DT)]
            wo_all = persist.tile([128, MT, DM], F16, tag="wo")
            wo_t = [wo_all[:, k, :] for k in range(MT)]

            def dma_block(src, n, nm, wtiles=None, wsrc=None):
                blk = xt.tile([128, DT, 512], BF16, tag="xt",
                              name=f"{nm}{n}")
                nc.sync.dma_start(
                    blk[:],
                    src[:, n * 512:(n + 1) * 512].rearrange(
                        "(k p) c -> p k c", p=128))
                if wtiles is not None:
                    nc.sync.dma_start(
                        wtiles,
                        wsrc[:, :].rearrange("(k p) c -> p k c", p=128))
                return [blk[:, k, :] for k in range(DT)]

            def proj_group(dst_tiles, w_sb, xts, n, m, pool, tag):
                ps = pool.tile([128, 512], F32, tag=tag, name=f"pj{n}_{m}_{tag}")
                for k in range(DT):
                    nc.tensor.matmul(
                        ps[:], w_sb[k][:, m * 128:(m + 1) * 128],
                        xts[k][:], start=(k == 0), stop=(k == DT - 1))
                nc.vector.tensor_copy(dst_tiles[(m, n)][:], ps[:])

            def proj_block(dst_tiles, w_sb, wt, src, n, nm, wsrc=None):
                xts = dma_block(src, n, nm,
                                wtiles=wt if wsrc is not None else None,
                                wsrc=wsrc)
                for m in range(MT):
                    proj_group(dst_tiles, w_sb, xts, n, m, psA, "pa")

            def v_group(n, sm, xts, pool, tag):
                t = n * 4 + sm
                ps = pool.tile([128, 512], F32, tag=tag, name=f"vps{n}_{sm}")
                for k in range(DT):
                    nc.tensor.matmul(
                        ps[:], xts[k][:, sm * 128:(sm + 1) * 128],
                        wv_sb[k][:], start=(k == 0), stop=(k == DT - 1))
                # evacuate with mask scaling; set+mask ones column
                nc.vector.tensor_scalar_mul(
                    v_sb[:, t, :, 0:DK],
                    ps[:].rearrange("p (h d) -> p h d", h=HLOC),
                    m_sb[:, t:t + 1])
                nc.vector.tensor_scalar_mul(
                    v_sb[:, t, :, DK:DK + 1], ones8[:],
                    m_sb[:, t:t + 1])

            def v_block(n, pool, tag, first=False):
                xts = dma_block(xvT, n, "xv",
                                wtiles=wv_t[:] if first else None,
                                wsrc=wvT if first else None)
                for sm in range(4):        # s-tiles within block
                    v_group(n, sm, xts, pool, tag)

            # PE warmup: dummy matmuls cover initial DMA latency and start
            # the HAM activity window before the first real matmul.
            dum = wq_pool.tile([128, 512], BF16, tag="dum")
            nc.vector.memset(dum[:], 0.0)
            for i in range(_env("K_WARM_MM", 14)):
                pw = psA.tile([128, 512], F32, tag="pa", name=f"warmmm{i}")
                nc.tensor.matmul(pw[:], dum[:, 0:128], dum[:],
                                 start=True, stop=True)
            proj_block(k_tiles, wk_sb, wk_t[:], xkT, 0, "xk", wsrc=wkT)
            for n in range(1, NB):
                proj_block(k_tiles, wk_sb, wk_t[:], xkT, n, "xk")
            proj_block(q_tiles, wq_sb, wq_t[:], xqT, 0, "xq", wsrc=wqT)
            v_block(0, psA, "pa", first=True)
            v_block(1, psA, "pa")
            nc.sync.dma_start(
                wo_all[:], woT[:, :].rearrange("(k p) c -> p k c", p=128))

        # ---------------- Phase B: attention + out-proj ----------------
        with tc.tile_pool(name="ev", bufs=_env("K_EV_BUFS", 36)) as ev, \
             tc.tile_pool(name="x", bufs=2) as xpool, \
             tc.tile_pool(name="small", bufs=_env("K_SMALL_BUFS", 4)) as small, \
             tc.tile_pool(name="o", bufs=2) as opool, \
             tc.tile_pool(name="psS", bufs=_env("K_PSS_BUFS", 3), space="PSUM") as psS, \
             tc.tile_pool(name="psX", bufs=1, space="PSUM") as psX:
            # one persistent PSUM accumulator; slice g = group (qc, hp).
            # Padded to a 128-f32 group stride so no slice straddles a
            # 2KB PSUM bank boundary (matmul outs must stay in one bank).
            px_all = psX.tile([128, 2 * MT, DK + 1], F32, tag="xo",
                              padded_shape=[128, 2 * MT, 128])
            # x in q-partition layout, per qt: 4 tiles [128 q, 512 hd] f16
            x_sb = [[xpool.tile([128, 512], F16, tag=f"xs{qc}",
                                name=f"xs{i}_{qc}") for qc in range(4)]
                    for i in range(2)]
            # x^T tiles for outproj, per qt: [128 hd, MT, 512 q] f16
            xT_tiles = [xpool.tile([128, MT, 512], F16, tag="xT",
                                   name=f"xT{i}") for i in range(2)]

            def outproj_group(oqt, m):
                xT_prev = xT_tiles[oqt % 2]
                po = psS.tile([128, 512], F32, tag="s", name=f"po{oqt}_{m}")
                for kk in range(MT):
                    nc.tensor.matmul(
                        po[:], wo_t[kk][:, m * 128:(m + 1) * 128],
                        xT_prev[:, kk, :], start=(kk == 0), stop=(kk == MT - 1))
                o_sb = opool.tile([128, 512], F32, tag="ob")
                nc.vector.tensor_copy(o_sb[:], po[:])
                nc.sync.dma_start(
                    outT[m * 128:(m + 1) * 128, oqt * 512:(oqt + 1) * 512],
                    o_sb[:])

            # side-work: one psS-slot matmul group (or a DMA batch) per sg
            # step. (qt0,p0): v-blocks 2,3. (qt0,p1..3): late q projections.
            # (qt>0,p): out-projection of qt-1.
            xts_store = {}

            def mk_vdma(nn):
                def f():
                    xts_store[("v", nn)] = dma_block(xvT, nn, "xv")
                return ("dma", f)

            def mk_vg(nn, sm):
                return ("mm", lambda: v_group(nn, sm, xts_store[("v", nn)],
                                              psS, "s"))

            def mk_qdma(nn):
                def f():
                    xts_store[("q", nn)] = dma_block(xqT, nn, "xq")
                return ("dma", f)

            def mk_qg(nn, m):
                return ("mm", lambda: proj_group(q_tiles, wq_sb,
                                                 xts_store[("q", nn)],
                                                 nn, m, psS, "s"))

            side_work = {}
            side_work[(0, 0)] = [
                (0, mk_vdma(2)), (1, mk_vdma(3)),
                (1, mk_vg(2, 0)), (2, mk_vg(2, 1)),
                (3, mk_vg(2, 2)), (4, mk_vg(2, 3)),
                (5, mk_vg(3, 0)), (6, mk_vg(3, 1)),
                (7, mk_vg(3, 2)), (99, mk_vg(3, 3)),
            ]
            QOFF = _env("K_QOFF", 3)
            for n in range(1, NB):
                side_work[(0, n)] = [(0, mk_qdma(n))] + [
                    (QOFF + m, mk_qg(n, m)) for m in range(MT)]

            # out-projection of qt-1 spread over pairs p=1..3 of qt (the
            # last transposes of (qt-1, 3) are only emitted during (qt, 0)).
            OPROJ_M = {1: (0, 1, 2), 2: (3, 4, 5), 3: (6, 7)}

            def side_step(qt, p, sg):
                if qt > 0 and p >= 1:
                    ms = OPROJ_M[p]
                    off = _env("K_OOFF", 1)
                    if sg >= off and (sg - off) % 2 == 0:
                        idx = (sg - off) // 2
                        if idx < len(ms):
                            outproj_group(qt - 1, ms[idx])
                    return
                work = side_work.get((qt, p))
                if not work:
                    return
                did_mm = False
                while work:
                    min_sg, (kind, fn) = work[0]
                    if min_sg > sg or (kind == "mm" and did_mm):
                        break
                    work.pop(0)
                    fn()
                    if kind == "mm":
                        did_mm = True

            def side_flush(qt, p):
                for _, (kind, fn) in side_work.pop((qt, p), []):
                    fn()

            # Pair pipeline: each slot of pair (qt,p) also runs one complete
            # attnV accumulation group of the PREVIOUS pair, plus its norm.
            # prev_pair state: (qt, p, e_tiles{h: [tile per sg]}).
            prev_pair = None

            def attnv_group(pqt, pp, e_tiles, g):
                """Emit accumulation group g (=(qc, hp)) of pair pp: 16
                matmuls into one psX slot, then normalize into x_sb."""
                qc, hp = divmod(g, 2)
                h = 2 * pp + hp
                px = px_all[:, g, :]
                for t in range(KT):
                    nc.tensor.matmul(
                        px,
                        e_tiles[hp][t // SGW][:, t % SGW,
                                              qc * 128:(qc + 1) * 128],
                        v_sb[:, t, h, :],
                        start=(t == 0), stop=(t == KT - 1))
                r = small.tile([128, 1], F32, tag="r",
                               name=f"r{pqt}_{pp}_{g}")
                nc.vector.reciprocal(r[:], px[:, DK:DK + 1])
                nc.vector.tensor_scalar(
                    x_sb[pqt % 2][qc][:, pp * 128 + hp * 64:
                                      pp * 128 + (hp + 1) * 64],
                    px[:, 0:DK], r[:], None, mybir.AluOpType.mult)

            def transpose_block(pqt, pp, qc):
                """x_sb[qc][:, pp*128:+128] -> xT[:, pp, qc*128:+128]."""
                pt = psS.tile([128, 128], F16, tag="s",
                              name=f"tr{pqt}_{pp}_{qc}")
                nc.tensor.transpose(
                    pt[:], x_sb[pqt % 2][qc][:, pp * 128:(pp + 1) * 128],
                    id16[:])
                nc.vector.tensor_copy(
                    xT_tiles[pqt % 2][:, pp, qc * 128:(qc + 1) * 128], pt[:])
                if xdbg is not None:
                    nc.sync.dma_start(
                        xdbg[pqt, :, pp, qc * 128:(qc + 1) * 128],
                        xT_tiles[pqt % 2][:, pp, qc * 128:(qc + 1) * 128])

            def drain_pair(pair, sg_from):
                """Emit remaining attnV groups + transposes of `pair`."""
                if pair is None:
                    return
                pqt, pp, e_tiles = pair
                for g in range(sg_from, NSG):
                    attnv_group(pqt, pp, e_tiles, g)
                for qc in range(4):
                    transpose_block(pqt, pp, qc)

            pair_list = [(qt, p) for qt in range(NB) for p in range(MT)]
            for qt, p in pair_list:
                heads = (2 * p, 2 * p + 1)
                e_tiles = {0: [None] * NSG, 1: [None] * NSG}
                for sg in range(NSG):
                    ps_s = {h: psS.tile([128, SGW, 512], F32, tag="s",
                                        name=f"pss{qt}_{sg}_{h}")
                            for h in heads}
                    # attnV group of the previous pair FIRST: it has no fresh
                    # dependencies, so it covers the semaphore latency of the
                    # scores' wait on exp(sg-1) draining its PSUM slot.
                    if prev_pair is not None:
                        attnv_group(prev_pair[0], prev_pair[1],
                                    prev_pair[2], sg)
                    side_step(qt, p, sg)
                    for tt in range(SGW):
                        t = sg * SGW + tt
                        for h in heads:
                            hp = h % 2
                            nc.tensor.matmul(
                                ps_s[h][:, tt, :],
                                k_tiles[(p, t // 4)][
                                    hp * 64:(hp + 1) * 64,
                                    (t % 4) * 128:(t % 4 + 1) * 128],
                                q_tiles[(p, qt)][hp * 64:(hp + 1) * 64, :],
                                start=True, stop=True)
                    # transposes of the previous pair trail the groups; last
                    # in the slot so the DVE norm they wait on has landed.
                    if prev_pair is not None:
                        if sg >= 2 and sg % 2 == 0:
                            transpose_block(prev_pair[0], prev_pair[1],
                                            sg // 2 - 1)
                        elif sg == NSG - 1:
                            transpose_block(prev_pair[0], prev_pair[1], 3)
                    # exp: ACT mostly; offload a half-tile (one head, tt=1)
                    # to DVE fast-exp on 2-of-3 slots to balance per-slot
                    # ACT time against PE (total offload mass = 1/6).
                    off_h = None
                    if EXPDVE_MOD == 3 and sg % 3:
                        off_h = heads[sg % 3 - 1]
                    elif EXPDVE_MOD > 3 and sg % EXPDVE_MOD == EXPDVE_MOD - 1:
                        off_h = "all"
                    for h in heads:
                        hp = h % 2
                        e_sb = ev.tile([128, SGW, 512], F16, tag="e",
                                       name=f"e{qt}_{p}_{sg}_{h}")
                        if off_h == "all":
                            nc.vector.tensor_scalar(
                                e_sb[:].bitcast(I16), ps_s[h][:],
                                FE_C1, FE_C2,
                                mybir.AluOpType.mult, mybir.AluOpType.add)
                        elif off_h == h:
                            nc.scalar.activation(e_sb[:, 0, :],
                                                 ps_s[h][:, 0, :], EXP,
                                                 scale=float(SCALE))
                            nc.vector.tensor_scalar(
                                e_sb[:, 1, :].bitcast(I16), ps_s[h][:, 1, :],
                                FE_C1, FE_C2,
                                mybir.AluOpType.mult, mybir.AluOpType.add)
                        else:
                            nc.scalar.activation(e_sb[:], ps_s[h][:], EXP,
                                                 scale=float(SCALE))
                        e_tiles[hp][sg] = e_sb
                side_flush(qt, p)
                prev_pair = (qt, p, e_tiles)
            # tail: drain the last pair and emit the final out-projection
            drain_pair(prev_pair, 0)
            for m in range(DT):
                outproj_group(NB - 1, m)
    nc.finalize()
    return nc


def kernel(query, key, value, mask, W_q, W_k, W_v, W_o):
    global _NC
    if _NC is None:
        _NC = _build()
    BF = ml_dtypes.bfloat16
    query = np.asarray(query, dtype=np.float32)
    key = np.asarray(key, dtype=np.float32)
    value = np.asarray(value, dtype=np.float32)
    W_q = np.asarray(W_q, dtype=np.float32)
    W_k = np.asarray(W_k, dtype=np.float32)
    W_v = np.asarray(W_v, dtype=np.float32)
    W_o = np.asarray(W_o, dtype=np.float32)
    mask = np.asarray(mask)
    ident = np.eye(128, dtype=np.float16)

    in_maps = []
    for c in range(NC_CORES):
        b, g = divmod(c, 2)
        hs = slice(g * CW, (g + 1) * CW)
        mrow = (mask[b, 0, 0, :] != 0).astype(np.float32)
        in_maps.append({
            "xqT": np.ascontiguousarray(query[b].T).astype(BF),
            "xkT": np.ascontiguousarray(key[b].T).astype(BF),
            "xvT": np.ascontiguousarray(value[b].T).astype(BF),
            "wqT": np.ascontiguousarray(W_q[hs, :].T).astype(BF),
            "wkT": np.ascontiguousarray(W_k[hs, :].T).astype(BF),
            "wvT": np.ascontiguousarray(W_v[hs, :].T).astype(BF),
            "woT": np.ascontiguousarray(W_o[:, hs].T).astype(np.float16),
            "maskf": np.ascontiguousarray(mrow.reshape(KT, 128).T),
            "ident": ident,
        })
    res = run_bass_kernel_spmd(_NC, in_maps, core_ids=list(range(NC_CORES)))
    out = np.empty((B, S, DM), np.float32)
    for b in range(B):
        out[b] = (res.results[2 * b]["outT"] + res.results[2 * b + 1]["outT"]).T
    return out
